# revision 1
# baseline (speedup 1.0000x reference)
"""DualAttention (position attention + channel attention) Trainium2 kernel.

Data-parallel over batch: 8 samples -> 8 NeuronCores, weights replicated.
All heavy matmuls run in bf16 (f32 PSUM accumulation); softmax math,
residual adds and the final output stay f32.

Self-contained: shapes/sharding hardcoded, no sibling imports.
"""

import numpy as np
import ml_dtypes
from contextlib import ExitStack

import concourse.bass as bass
import concourse.tile as tile
from concourse import bacc, mybir
from concourse.bass_utils import run_bass_kernel_spmd
from concourse.masks import make_identity

F32 = mybir.dt.float32
BF16 = mybir.dt.bfloat16
AF = mybir.ActivationFunctionType
OP = mybir.AluOpType
AX = mybir.AxisListType
NPBF = ml_dtypes.bfloat16

EPS = 1e-5
P = 2048      # positions
CIN = 512     # input channels (4 chunks of 128)
CI = 128      # inner channels
CQ = 16       # q/k channels
COUT = 512    # output channels (4 chunks of 128)
NCORES = 8
NJC = P // 128   # 16 j-chunks / p-subtiles


def _build_module():
    nc = bacc.Bacc("TRN2", target_bir_lowering=False, debug=False,
                   num_devices=NCORES)

    # ---------------- DRAM I/O ----------------
    dx = nc.dram_tensor("x", [128, 4, P], BF16, kind="ExternalInput")
    dw5a = nc.dram_tensor("w5a", [128, 12, 128], BF16, kind="ExternalInput")
    db5a = nc.dram_tensor("b5a", [128, 1], F32, kind="ExternalInput")
    dw5c = nc.dram_tensor("w5c", [128, 12, 128], BF16, kind="ExternalInput")
    db5c = nc.dram_tensor("b5c", [128, 1], F32, kind="ExternalInput")
    dwq4 = nc.dram_tensor("wq4", [128, 128], BF16, kind="ExternalInput")
    dwk4 = nc.dram_tensor("wk4", [128, 128], BF16, kind="ExternalInput")
    dbq4 = nc.dram_tensor("bq4", [128, 1], F32, kind="ExternalInput")
    dbk4 = nc.dram_tensor("bk4", [128, 1], F32, kind="ExternalInput")
    dwv = nc.dram_tensor("wv", [128, 128], BF16, kind="ExternalInput")
    dw51 = nc.dram_tensor("w51", [128, 3, 128], BF16, kind="ExternalInput")
    db51 = nc.dram_tensor("b51", [128, 1], F32, kind="ExternalInput")
    dw52 = nc.dram_tensor("w52", [128, 3, 128], BF16, kind="ExternalInput")
    db52 = nc.dram_tensor("b52", [128, 1], F32, kind="ExternalInput")
    dw8 = nc.dram_tensor("w8", [128, 4, 128], BF16, kind="ExternalInput")
    db8 = nc.dram_tensor("b8", [128, 4], F32, kind="ExternalInput")
    dalpa = nc.dram_tensor("alpa", [128, 1], F32, kind="ExternalInput")
    dabpa = nc.dram_tensor("abpa", [128, 1], F32, kind="ExternalInput")
    dalca = nc.dram_tensor("alca", [128, 1], F32, kind="ExternalInput")
    dout = nc.dram_tensor("out", [4, 128, P], F32, kind="ExternalOutput")

    with tile.TileContext(nc) as tc, ExitStack() as ctx:
        const = ctx.enter_context(tc.tile_pool(name="const", bufs=1))
        feats = ctx.enter_context(tc.tile_pool(name="feats", bufs=1))
        expsp = ctx.enter_context(tc.tile_pool(name="expsp", bufs=NJC))
        outp = ctx.enter_context(tc.tile_pool(name="outp", bufs=2))
        smallp = ctx.enter_context(tc.tile_pool(name="smallp", bufs=4))
        # PSUM: st 2x[128,2048]bf16 (4 banks) + cc 2x[128,512]f32 (2 banks)
        #       + tp 1x[128,128]f32 (1 bank) + e2 1x[128,128]f32 (1 bank)
        pst = ctx.enter_context(tc.tile_pool(name="pst", bufs=2, space="PSUM"))
        pcc = ctx.enter_context(tc.tile_pool(name="pcc", bufs=2, space="PSUM"))
        ptp = ctx.enter_context(tc.tile_pool(name="ptp", bufs=1, space="PSUM"))
        pe2 = ctx.enter_context(tc.tile_pool(name="pe2", bufs=1, space="PSUM"))

        # ---------------- constants in ----------------
        _dma_rr = [nc.sync, nc.sync]
        _dma_i = [0]

        def cload(name, shape, dtype, dram):
            t = const.tile(shape, dtype, tag=name)
            eng = _dma_rr[_dma_i[0] % len(_dma_rr)]
            _dma_i[0] += 1
            eng.dma_start(t[:], dram[:])
            return t

        w5a = const.tile([128, 12, 128], BF16, tag="w5a")
        x_sb = const.tile([128, 4, P], BF16, tag="x")
        # DMA dispatch costs ~0.65us of sequencer time each; spread the head
        # transfers across otherwise-idle sequencers so the first conv
        # operands land as early as possible.
        nc.sync.dma_start(w5a[:, 0:6, :], dw5a[:, 0:6, :])
        nc.gpsimd.dma_start(x_sb[:, 0, 0:516], dx[:, 0, 0:516])
        nc.gpsimd.dma_start(x_sb[:, 1, 0:516], dx[:, 1, 0:516])
        nc.sync.dma_start(w5a[:, 6:12, :], dw5a[:, 6:12, :])
        nc.sync.dma_start(x_sb[:, 2, 0:516], dx[:, 2, 0:516])
        nc.sync.dma_start(x_sb[:, 3, 0:516], dx[:, 3, 0:516])
        b5a = cload("b5a", [128, 1], F32, db5a)
        xsplit = [516, 1028, 1540, 2048]
        xeng = [nc.sync, nc.sync, nc.sync]
        for r in range(3):
            xeng[r].dma_start(x_sb[:, :, xsplit[r]:xsplit[r + 1]],
                              dx[:, :, xsplit[r]:xsplit[r + 1]])
        wq4 = cload("wq4", [128, 128], BF16, dwq4)
        wk4 = cload("wk4", [128, 128], BF16, dwk4)
        bq4 = cload("bq4", [128, 1], F32, dbq4)
        bk4 = cload("bk4", [128, 1], F32, dbk4)
        wv = cload("wv", [128, 128], BF16, dwv)
        abpa = cload("abpa", [128, 1], F32, dabpa)
        w5c = cload("w5c", [128, 12, 128], BF16, dw5c)
        b5c = cload("b5c", [128, 1], F32, db5c)
        w51 = cload("w51", [128, 3, 128], BF16, dw51)
        b51 = cload("b51", [128, 1], F32, db51)
        w52 = cload("w52", [128, 3, 128], BF16, dw52)
        b52 = cload("b52", [128, 1], F32, db52)
        w8 = cload("w8", [128, 4, 128], BF16, dw8)
        b8 = cload("b8", [128, 4], F32, db8)
        alpa = cload("alpa", [128, 1], F32, dalpa)
        alca = cload("alca", [128, 1], F32, dalca)

        ident = const.tile([128, 128], BF16, tag="ident")
        make_identity(nc, ident[:])

        # persistent feature tiles
        feat1_f = feats.tile([128, P], F32, tag="feat1_f")
        feat1_b = feats.tile([128, P], BF16, tag="feat1_b")
        feat1_a = feats.tile([128, P], F32, tag="feat1_a")  # feat1 + alpha*vb
        feat2_f = feats.tile([128, P], F32, tag="feat2_f")
        feat2_b = feats.tile([128, P], BF16, tag="feat2_b")
        q_rep = feats.tile([128, P], BF16, tag="q_rep")
        k_rep = feats.tile([128, P], BF16, tag="k_rep")
        vt_all = feats.tile([128, NJC, 130], BF16, tag="vt_all")
        f2t_all = feats.tile([128, NJC, 128], BF16, tag="f2t_all")
        sa_feat = feats.tile([128, P], BF16, tag="sa_feat")
        sc_feat = feats.tile([128, P], BF16, tag="sc_feat")
        sa_conv = feats.tile([128, P], BF16, tag="sa_conv")
        sc_conv = feats.tile([128, P], BF16, tag="sc_conv")
        feat_sum = feats.tile([128, P], BF16, tag="feat_sum")

        # ---------------- helpers ----------------
        def conv3_block(psum, rhs2d_list, w_sb, b0, W=512):
            """3-tap conv over output cols [b0, b0+W) into psum [128,W].
            rhs2d_list: list of [128,P] source APs (cin chunks).
            w_sb: [128, 3*nchunks, 128] lhsT per (chunk, tap)."""
            nch = len(rhs2d_list)
            first = True
            for s in (0, -1, 1):
                ol = max(b0, 1) if s == -1 else b0
                oh = min(b0 + W, P - 1) if s == 1 else b0 + W
                for c in range(nch):
                    last = (s == 1 and c == nch - 1)
                    nc.tensor.matmul(
                        psum[:, ol - b0:oh - b0],
                        w_sb[:, c * 3 + (s + 1), :],
                        rhs2d_list[c][:, ol + s:oh + s],
                        start=first, stop=last)
                    first = False

        xs = [x_sb[:, c, :] for c in range(4)]

        # warm the ACT exp table off the critical path (first Exp use
        # triggers a ~2.7us table load)
        warm = smallp.tile([128, 1], F32, tag="warm")
        nc.scalar.activation(warm[:], ident[:, 0:1], AF.Exp)
        nc.vector.memset(vt_all[:, :, 128:130], 1.0)

        # ---- Phase A: conv5a + qk, interleaved so q/k h0 is ready early ---
        def conv5a_block(b):
            ps = pcc.tile([128, 512], F32, tag="cc")
            conv3_block(ps, xs, w5a, b * 512)
            sl = slice(b * 512, (b + 1) * 512)
            nc.scalar.activation(feat1_f[:, sl], ps[:], AF.Relu, bias=b5a[:])
            nc.gpsimd.tensor_copy(feat1_b[:, sl], feat1_f[:, sl])

        def qk_half(h):
            # q and k each replicated to partition rows {0:16, 64:80} so the
            # S_T matmuls can run 2-way row-tiled (strips (0,0) and (64,0))
            sl = slice(h * 1024, (h + 1) * 1024)
            psq = pst.tile([128, 1024], F32, tag="st")
            for i in range(2):
                o = h * 1024 + i * 512
                nc.tensor.matmul(psq[:, i * 512:(i + 1) * 512], wq4[:],
                                 feat1_b[:, o:o + 512], start=True, stop=True)
            for i in range(2):
                o = h * 1024 + i * 512
                nc.vector.tensor_scalar_add(q_rep[:, o:o + 512],
                                            psq[:, i * 512:(i + 1) * 512],
                                            bq4[:])
            psk = pst.tile([128, 1024], F32, tag="st")
            for i in range(2):
                o = h * 1024 + i * 512
                nc.tensor.matmul(psk[:, i * 512:(i + 1) * 512], wk4[:],
                                 feat1_b[:, o:o + 512], start=True, stop=True)
            for i in range(2):
                o = h * 1024 + i * 512
                nc.scalar.activation(k_rep[:, o:o + 512],
                                     psk[:, i * 512:(i + 1) * 512],
                                     AF.Identity, bias=bk4[:])

        conv5a_block(0)
        conv5a_block(1)
        qk_half(0)
        conv5a_block(2)
        conv5a_block(3)
        qk_half(1)
        # feat1 + alpha*vb (for the position-attention residual epilogue)
        nc.vector.tensor_scalar_add(feat1_a[:], feat1_f[:], abpa[:])

        # ---------------- window filler units -----------------------------
        units = []

        def u_vt(g):
            # vT[p,c] = feat1.T @ wv^T ; 4 p-subs per psum tile
            def f():
                ps = pcc.tile([128, 512], F32, tag="cc")
                for i in range(4):
                    sub = g * 4 + i
                    nc.tensor.matmul(ps[:, i * 128:(i + 1) * 128],
                                     feat1_b[:, sub * 128:(sub + 1) * 128],
                                     wv[:], start=True, stop=True)
                nc.any.tensor_copy(vt_all[:, g * 4:(g + 1) * 4, 0:128],
                                   ps[:].rearrange("p (s c) -> p s c", s=4))
            return f

        def u_conv5c(hb):
            def f():
                ps = pcc.tile([128, 256], F32, tag="cc")
                conv3_block(ps, xs, w5c, hb * 256, W=256)
                sl = slice(hb * 256, (hb + 1) * 256)
                nc.vector.tensor_scalar(feat2_f[:, sl], ps[:], b5c[:], 0.0,
                                        op0=OP.add, op1=OP.max)
                nc.gpsimd.tensor_copy(feat2_b[:, sl], feat2_f[:, sl])
            return f

        e2_ps = pe2.tile([128, 128], F32, tag="e2")

        def u_f2t(g):
            def f():
                ps = pcc.tile([128, 512], BF16, tag="cc")
                for i in range(4):
                    sub = g * 4 + i
                    nc.tensor.transpose(ps[:, i * 128:(i + 1) * 128],
                                        feat2_b[:, sub * 128:(sub + 1) * 128],
                                        ident[:])
                nc.any.tensor_copy(f2t_all[:, g * 4:(g + 1) * 4, :],
                                   ps[:].rearrange("p (s c) -> p s c", s=4))
                # channel-attention gram accumulation for this group
                for i in range(4):
                    sub = g * 4 + i
                    nc.tensor.matmul(e2_ps[:], f2t_all[:, sub, :],
                                     f2t_all[:, sub, :],
                                     start=(sub == 0), stop=(sub == NJC - 1))
            return f

        attn2 = feats.tile([128, 128], BF16, tag="attn2")
        attn2n = feats.tile([128, 128], BF16, tag="attn2n")
        a2t = feats.tile([128, 128], BF16, tag="a2t")

        def u_softmax2():
            rmin = smallp.tile([128, 1], F32, tag="rmin")
            den2 = smallp.tile([128, 1], F32, tag="den2")
            rden2 = smallp.tile([128, 1], F32, tag="rden2")
            # softmax(max-E) == exp(min-E)/sum: exp(-E + rowmin)
            nc.vector.tensor_reduce(rmin[:], e2_ps[:], axis=AX.X, op=OP.min)
            nc.scalar.activation(attn2[:], e2_ps[:], AF.Exp, bias=rmin[:],
                                 scale=-1.0, accum_out=den2[:])
            nc.vector.reciprocal(rden2[:], den2[:])
            nc.any.tensor_scalar_mul(attn2n[:], attn2[:], rden2[:])
            pt = ptp.tile([128, 128], BF16, tag="tp")
            nc.tensor.transpose(pt[:], attn2n[:], ident[:])
            nc.any.tensor_copy(a2t[:], pt[:])

        def u_out2(b):
            def f():
                ps = pcc.tile([128, 512], F32, tag="cc")
                nc.tensor.matmul(ps[:], a2t[:],
                                 feat2_b[:, b * 512:(b + 1) * 512],
                                 start=True, stop=True)
                # sc_feat = ca_alpha*out2 + feat2
                nc.vector.scalar_tensor_tensor(
                    sc_feat[:, b * 512:(b + 1) * 512], ps[:], alca[:],
                    feat2_f[:, b * 512:(b + 1) * 512], op0=OP.mult, op1=OP.add)
            return f

        def u_c52(b):
            def f():
                ps = pcc.tile([128, 512], F32, tag="cc")
                conv3_block(ps, [sc_feat[:]], w52, b * 512)
                nc.vector.tensor_scalar(sc_conv[:, b * 512:(b + 1) * 512],
                                        ps[:], b52[:], 0.0,
                                        op0=OP.add, op1=OP.max)
            return f

        def u_c51w(o0):
            # in-window c51 block: psum from cc, relu+add on DVE (ACT is the
            # window bottleneck); needs sa_feat cols <= o0+512+1
            def f():
                sl = slice(o0, o0 + 512)
                ps = pcc.tile([128, 512], F32, tag="cc")
                conv3_block(ps, [sa_feat[:]], w51, o0)
                nc.vector.tensor_scalar(sa_conv[:, sl], ps[:], b51[:], 0.0,
                                        op0=OP.add, op1=OP.max)
                nc.vector.tensor_add(feat_sum[:, sl], sa_conv[:, sl],
                                     sc_conv[:, sl])
            return f

        def u_c8w(o0, co):
            def f():
                sl = slice(o0, o0 + 512)
                p8 = pcc.tile([128, 512], F32, tag="cc")
                nc.tensor.matmul(p8[:], w8[:, co, :], feat_sum[:, sl],
                                 start=True, stop=True)
                ot = outp.tile([128, 512], F32, tag="out_sb", bufs=6)
                nc.vector.tensor_scalar_add(ot[:], p8[:], b8[:, co:co + 1])
                nc.sync.dma_start(dout[co, :, sl], ot[:])
            return f

        for hb in range(8):
            units.append((u_conv5c(hb), 800))
            if hb < 4:
                units.append((u_vt(hb), 600))
        for g in range(4):
            units.append((u_f2t(g), 600))
        units.append((u_softmax2, 300))
        for b in range(4):
            units.append((u_out2(b), 250))
        for b in range(4):
            units.append((u_c52(b), 700))
        units.append((u_c51w(0), 1000))
        for co in range(4):
            units.append((u_c8w(0, co), 600))
        units.append((u_c51w(512), 1000))
        for co in range(4):
            units.append((u_c8w(512, co), 600))

        # ---------------- AV emitter (used in window + after) -------------
        def emit_av(isub):
            ps = pcc.tile([128, 132], F32, tag="cc")
            for jc in range(NJC):
                est = es2[(jc // 2) * 4 + isub // 4]
                off = (jc % 2) * 512 + (isub % 4) * 128
                nc.tensor.matmul(ps[:, 0:129],
                                 est[:, off:off + 128],
                                 vt_all[:, jc, 0:129],
                                 start=(jc == 0), stop=(jc == NJC - 1))
            rcol = smallp.tile([128, 1], F32, tag="rcol", bufs=8)
            nc.vector.reciprocal(rcol[:], ps[:, 128:129])
            onrm = smallp.tile([128, 128], BF16, tag="onrm", bufs=4)
            nc.any.tensor_scalar_mul(onrm[:], ps[:, 0:128], rcol[:])
            tpool = ptp if isub % 2 == 0 else pe2
            ttag = "tp" if isub % 2 == 0 else "e2"
            pt = tpool.tile([128, 128], BF16, tag=ttag)
            nc.tensor.transpose(pt[:], onrm[:], ident[:])
            # sa_feat = alpha*outT + (feat1 + alpha*vb)
            nc.vector.scalar_tensor_tensor(
                sa_feat[:, isub * 128:(isub + 1) * 128], pt[:], alpa[:],
                feat1_a[:, isub * 128:(isub + 1) * 128],
                op0=OP.mult, op1=OP.add)

        # ---------------- Phase B: S^T + exp window -----------------------
        # S_T[j, i] = sum_d k[d,j] q[d,i]; exp -> expS (bf16).
        # 2-way row-tiled: strips (0,0)/(64,0) compute jc pair (2t, 2t+1)
        # concurrently. i-block-major order so AV isubs start mid-window.
        # es2[t*4+b]: [128, 0:512]=expS[2t][:, b*512:], [512:]=expS[2t+1].
        es2 = [None] * 32
        # step order: first 8 steps need only the h0 halves of q/k (pairs
        # t<4 x columns b<2), so exps start while conv5a b2/b3 + qk h1 are
        # still in flight; then b-major for the rest.
        order = [(t, b) for t in range(4) for b in range(2)]
        seen = set(order)
        for b in range(4):
            for t in range(8):
                if (t, b) not in seen:
                    order.append((t, b))
        colcnt = [0] * 4
        av_next = 0
        for (t, b) in order:
            es = expsp.tile([128, 1024], BF16, tag="expS",
                            name=f"es{t}_{b}")
            es2[t * 4 + b] = es
            ps = pst.tile([128, 1024], F32, tag="st")
            jc0, jc1 = 2 * t, 2 * t + 1
            bb = slice(b * 512, (b + 1) * 512)
            nc.tensor.matmul(ps[:, 0:512],
                             k_rep[0:16, jc0 * 128:(jc0 + 1) * 128],
                             q_rep[0:16, bb], start=True, stop=True,
                             tile_position=(0, 0))
            nc.tensor.matmul(ps[:, 512:1024],
                             k_rep[64:80, jc1 * 128:(jc1 + 1) * 128],
                             q_rep[64:80, bb], start=True, stop=True,
                             tile_position=(64, 0))
            nc.scalar.activation(es[:], ps[:], AF.Exp)
            colcnt[b] += 1
            # keep the PE just behind the ACT exp rate (~1us/step)
            budget = 300.0
            while units and budget > 0:
                f, cost = units.pop(0)
                f()
                budget -= cost
            # AV isubs for completed i-columns ride inside the window
            if av_next < 12 and colcnt[av_next // 4] == 8:
                emit_av(av_next)
                av_next += 1
        # avs 12/13 first: their early MMs depend on es2 tiles finished
        # several window steps ago, so they overlap the window tail; the
        # few leftover units follow in the stream.
        emit_av(12)
        emit_av(13)
        while units:
            units.pop(0)[0]()

        # ------- Phase C/D: AV isubs 8..15 + tail woven in ----------------
        def t_conv(o0, W=512):
            """c51 cols [o0, o0+W) -> feat_sum (ACT relu: ACT is idle here)."""
            sl = slice(o0, o0 + W)
            ps = pst.tile([128, 512], F32, tag="st")
            conv3_block(ps[:, 0:W], [sa_feat[:]], w51, o0, W=W)
            nc.scalar.activation(sa_conv[:, sl], ps[:, 0:W], AF.Relu,
                                 bias=b51[:])
            nc.vector.tensor_add(feat_sum[:, sl], sa_conv[:, sl],
                                 sc_conv[:, sl])

        def t_c8(o0, co, W=512):
            sl = slice(o0, o0 + W)
            p8 = pst.tile([128, 512], F32, tag="st")
            nc.tensor.matmul(p8[:, 0:W], w8[:, co, :], feat_sum[:, sl],
                             start=True, stop=True)
            ot = outp.tile([128, 512], F32, tag="out_sb", bufs=6)
            nc.any.tensor_scalar_add(ot[:, 0:W], p8[:, 0:W], b8[:, co:co + 1])
            nc.sync.dma_start(dout[co, :, sl], ot[:, 0:W])

        # c51 cols [o, o+W) need sa_feat cols <= o+W, i.e. isubs <= (o+W)/128
        # (isubs 0..11 completed inside the window)
        t_conv(1024)
        emit_av(14)
        t_c8(1024, 0)
        emit_av(15)
        t_c8(1024, 1)
        t_c8(1024, 2)
        t_c8(1024, 3)
        # final c51 block in engine-parallel halves: ACT does one relu while
        # DVE does the other; adds on DVE/gpsimd — shortens the last chain
        slA = slice(1536, 1792)
        psA = pst.tile([128, 512], F32, tag="st", name="c51fA")
        conv3_block(psA[:, 0:256], [sa_feat[:]], w51, 1536, W=256)
        nc.scalar.activation(sa_conv[:, slA], psA[:, 0:256], AF.Relu,
                             bias=b51[:])
        nc.vector.tensor_add(feat_sum[:, slA], sa_conv[:, slA],
                             sc_conv[:, slA])
        slB = slice(1792, 2048)
        psB = pcc.tile([128, 512], F32, tag="cc", name="c51fB")
        conv3_block(psB[:, 0:256], [sa_feat[:]], w51, 1792, W=256)
        nc.vector.tensor_scalar(sa_conv[:, slB], psB[:, 0:256], b51[:], 0.0,
                                op0=OP.add, op1=OP.max)
        nc.vector.tensor_add(feat_sum[:, slB], sa_conv[:, slB],
                             sc_conv[:, slB])
        t_c8(1536, 0)
        t_c8(1536, 1)
        t_c8(1536, 2)
        t_c8(1536, 3)

    nc.compile()
    return nc


_NC = None


def _get_nc():
    global _NC
    if _NC is None:
        _NC = _build_module()
    return _NC


def _wrep(w):
    z = np.zeros((128, 128), np.float32)
    z[:, 0:16] = w[:, :, 0].T
    z[:, 64:80] = w[:, :, 0].T
    return z


def _brep(b):
    z = np.zeros((128, 1), np.float32)
    z[0:16, 0] = b
    z[64:80, 0] = b
    return z


def _prep_inputs(inputs):
    """Host-side: fold BN into conv weights, transpose to lhsT layouts,
    cast matmul operands to bf16. Returns (shared_map, per_core_x)."""
    f32 = np.float32

    def fold(w, g, b, m, v):
        s = (g / np.sqrt(v + EPS)).astype(f32)
        return (w * s[:, None, None]).astype(f32), (b - m * s).astype(f32)

    w5a, b5a = fold(inputs['c5a_w'], inputs['c5a_g'], inputs['c5a_b'],
                    inputs['c5a_m'], inputs['c5a_v'])
    w5c, b5c = fold(inputs['c5c_w'], inputs['c5c_g'], inputs['c5c_b'],
                    inputs['c5c_m'], inputs['c5c_v'])
    w51, b51 = fold(inputs['c51_w'], inputs['c51_g'], inputs['c51_b'],
                    inputs['c51_m'], inputs['c51_v'])
    w52, b52 = fold(inputs['c52_w'], inputs['c52_g'], inputs['c52_b'],
                    inputs['c52_m'], inputs['c52_v'])

    def big_lhsT(w):  # [128, 512, 3] -> [p, chunk*3+tap, c] = [128, 12, 128]
        return np.ascontiguousarray(
            w.reshape(128, 4, 128, 3).transpose(2, 1, 3, 0)
        ).reshape(128, 12, 128)

    def small_lhsT(w):  # [128, 128, 3] -> [p, tap, c] = [128, 3, 128]
        return np.ascontiguousarray(w.transpose(1, 2, 0))

    pa = float(np.asarray(inputs['pa_alpha']).reshape(-1)[0])
    ca = float(np.asarray(inputs['ca_alpha']).reshape(-1)[0])

    shared = {
        'w5a': big_lhsT(w5a).astype(NPBF),
        'b5a': b5a.reshape(128, 1),
        'w5c': big_lhsT(w5c).astype(NPBF),
        'b5c': b5c.reshape(128, 1),
        'wq4': _wrep(inputs['qw']).astype(NPBF),
        'wk4': _wrep(inputs['kw']).astype(NPBF),
        'bq4': _brep(inputs['qb']).astype(f32),
        'bk4': _brep(inputs['kb']).astype(f32),
        'wv': np.ascontiguousarray(inputs['vw'][:, :, 0].T).astype(NPBF),
        'w51': small_lhsT(w51).astype(NPBF),
        'b51': b51.reshape(128, 1),
        'w52': small_lhsT(w52).astype(NPBF),
        'b52': b52.reshape(128, 1),
        'w8': np.ascontiguousarray(
            inputs['c8_w'][:, :, 0].reshape(4, 128, 128).transpose(2, 0, 1)
        ).astype(NPBF),
        'b8': np.ascontiguousarray(
            inputs['c8_b'].reshape(4, 128).T).astype(f32),
        'alpa': np.full((128, 1), pa, f32),
        'abpa': (pa * np.asarray(inputs['vb'])).reshape(128, 1).astype(f32),
        'alca': np.full((128, 1), ca, f32),
    }
    shared = {k: np.ascontiguousarray(v) for k, v in shared.items()}

    x = np.asarray(inputs['x'])  # [8, 512, 2048]
    per_core_x = [
        np.ascontiguousarray(
            x[b].reshape(4, 128, P).transpose(1, 0, 2).astype(NPBF))
        for b in range(NCORES)
    ]
    return shared, per_core_x


def kernel(**inputs) -> np.ndarray:
    inputs = {k: np.asarray(v) for k, v in inputs.items()}
    nc = _get_nc()
    shared, per_core_x = _prep_inputs(inputs)
    in_maps = [dict(shared, x=per_core_x[b]) for b in range(NCORES)]
    last_err = None
    for _attempt in range(3):
        try:
            res = run_bass_kernel_spmd(nc, in_maps,
                                       core_ids=list(range(NCORES)))
            break
        except Exception as e:  # transient device errors: retry
            last_err = e
            import time as _time
            _time.sleep(2.0)
    else:
        raise last_err
    out = np.stack([res.results[b]['out'].reshape(COUT, P)
                    for b in range(NCORES)])
    return out.astype(np.float32)



# revision 24
# speedup vs baseline: 1.1276x; 1.1276x over previous
"""DualAttention (position attention + channel attention) Trainium2 kernel.

Data-parallel over batch: 8 samples -> 8 NeuronCores, weights replicated.
All heavy matmuls run in bf16 (f32 PSUM accumulation); softmax math,
residual adds and the final output stay f32.

Self-contained: shapes/sharding hardcoded, no sibling imports.
"""

import numpy as np
import ml_dtypes
from contextlib import ExitStack

import concourse.bass as bass
import concourse.tile as tile
from concourse import bacc, mybir
from concourse.bass_utils import run_bass_kernel_spmd
from concourse.masks import make_identity

F32 = mybir.dt.float32
BF16 = mybir.dt.bfloat16
F8 = mybir.dt.float8e4
DR = mybir.MatmulPerfMode.DoubleRow
AF = mybir.ActivationFunctionType
OP = mybir.AluOpType
AX = mybir.AxisListType
NPBF = ml_dtypes.bfloat16
NPF8 = ml_dtypes.float8_e4m3

EPS = 1e-5
SC = 16.0     # fp8 weight scale (keeps w*16 in e4m3 normal range)
RSC = 1.0 / SC
P = 2048      # positions
CIN = 512     # input channels (4 chunks of 128)
CI = 128      # inner channels
CQ = 16       # q/k channels
COUT = 512    # output channels (4 chunks of 128)
NCORES = 8
NJC = P // 128   # 16 j-chunks / p-subtiles


def _build_module():
    nc = bacc.Bacc("TRN2", target_bir_lowering=False, debug=False,
                   num_devices=NCORES)

    # ---------------- DRAM I/O ----------------
    dx = nc.dram_tensor("x", [128, 4, P], BF16, kind="ExternalInput")
    dw5a = nc.dram_tensor("w5a", [128, 12, 128], BF16, kind="ExternalInput")
    db5a = nc.dram_tensor("b5a", [128, 1], F32, kind="ExternalInput")
    dw5c = nc.dram_tensor("w5c", [128, 12, 128], BF16, kind="ExternalInput")
    db5c = nc.dram_tensor("b5c", [128, 1], F32, kind="ExternalInput")
    dwq3 = nc.dram_tensor("wq3", [128, 3, CQ], F8, kind="ExternalInput")
    dwk3 = nc.dram_tensor("wk3", [128, 3, CQ], F8, kind="ExternalInput")
    dbq = nc.dram_tensor("bq", [CQ, 1], F32, kind="ExternalInput")
    dbk = nc.dram_tensor("bk", [CQ, 1], F32, kind="ExternalInput")
    dwv3 = nc.dram_tensor("wv3", [128, 3, 128], F8, kind="ExternalInput")
    dw51 = nc.dram_tensor("w51", [128, 3, 128], BF16, kind="ExternalInput")
    db51 = nc.dram_tensor("b51", [128, 1], F32, kind="ExternalInput")
    dw52 = nc.dram_tensor("w52", [128, 3, 128], BF16, kind="ExternalInput")
    db52 = nc.dram_tensor("b52", [128, 1], F32, kind="ExternalInput")
    dw8 = nc.dram_tensor("w8", [128, 4, 128], BF16, kind="ExternalInput")
    db8 = nc.dram_tensor("b8", [128, 4], F32, kind="ExternalInput")
    dalpa = nc.dram_tensor("alpa", [128, 1], F32, kind="ExternalInput")
    dabpa = nc.dram_tensor("abpa", [128, 1], F32, kind="ExternalInput")
    dalca = nc.dram_tensor("alca", [128, 1], F32, kind="ExternalInput")
    dout = nc.dram_tensor("out", [4, 128, P], BF16, kind="ExternalOutput")

    with tile.TileContext(nc) as tc, ExitStack() as ctx:
        const = ctx.enter_context(tc.tile_pool(name="const", bufs=1))
        feats = ctx.enter_context(tc.tile_pool(name="feats", bufs=1))
        expsp = ctx.enter_context(tc.tile_pool(name="expsp", bufs=32))
        outp = ctx.enter_context(tc.tile_pool(name="outp", bufs=2))
        smallp = ctx.enter_context(tc.tile_pool(name="smallp", bufs=4))
        # PSUM: st 2x[128,2048]bf16 (4 banks) + cc 2x[128,512]f32 (2 banks)
        #       + tp 1x[128,128]f32 (1 bank) + e2 1x[128,128]f32 (1 bank)
        pst = ctx.enter_context(tc.tile_pool(name="pst", bufs=2, space="PSUM"))
        pcc = ctx.enter_context(tc.tile_pool(name="pcc", bufs=2, space="PSUM"))
        ptp = ctx.enter_context(tc.tile_pool(name="ptp", bufs=1, space="PSUM"))
        pe2 = ctx.enter_context(tc.tile_pool(name="pe2", bufs=1, space="PSUM"))

        # ---------------- constants in ----------------
        _dma_rr = [nc.sync, nc.sync]
        _dma_i = [0]

        def cload(name, shape, dtype, dram):
            t = const.tile(shape, dtype, tag=name)
            eng = _dma_rr[_dma_i[0] % len(_dma_rr)]
            _dma_i[0] += 1
            eng.dma_start(t[:], dram[:])
            return t

        w5a = const.tile([128, 12, 128], BF16, tag="w5a")
        x_sb = const.tile([128, 4, P], BF16, tag="x")
        # DMA dispatch costs ~0.65us of sequencer time each; spread the head
        # transfers across otherwise-idle sequencers so the first conv
        # operands land as early as possible.
        nc.sync.dma_start(w5a[:, 0:6, :], dw5a[:, 0:6, :])
        nc.gpsimd.dma_start(x_sb[:, 0, 0:516], dx[:, 0, 0:516])
        nc.gpsimd.dma_start(x_sb[:, 1, 0:516], dx[:, 1, 0:516])
        nc.sync.dma_start(w5a[:, 6:12, :], dw5a[:, 6:12, :])
        nc.sync.dma_start(x_sb[:, 2, 0:516], dx[:, 2, 0:516])
        nc.sync.dma_start(x_sb[:, 3, 0:516], dx[:, 3, 0:516])
        b5a = cload("b5a", [128, 1], F32, db5a)
        xsplit = [516, 1028, 1540, 2048]
        xeng = [nc.sync, nc.sync, nc.sync]
        for r in range(3):
            xeng[r].dma_start(x_sb[:, :, xsplit[r]:xsplit[r + 1]],
                              dx[:, :, xsplit[r]:xsplit[r + 1]])
        wq3 = cload("wq3", [128, 3, CQ], F8, dwq3)
        wk3 = cload("wk3", [128, 3, CQ], F8, dwk3)
        bq = cload("bq", [CQ, 1], F32, dbq)
        bk = cload("bk", [CQ, 1], F32, dbk)
        wv3 = cload("wv3", [128, 3, 128], F8, dwv3)
        abpa = cload("abpa", [128, 1], F32, dabpa)
        w5c = cload("w5c", [128, 12, 128], BF16, dw5c)
        b5c = cload("b5c", [128, 1], F32, db5c)
        w51 = cload("w51", [128, 3, 128], BF16, dw51)
        b51 = cload("b51", [128, 1], F32, db51)
        w52 = cload("w52", [128, 3, 128], BF16, dw52)
        b52 = cload("b52", [128, 1], F32, db52)
        w8 = cload("w8", [128, 4, 128], BF16, dw8)
        b8 = cload("b8", [128, 4], F32, db8)
        alpa = cload("alpa", [128, 1], F32, dalpa)
        alca = cload("alca", [128, 1], F32, dalca)

        ident = const.tile([128, 128], BF16, tag="ident")
        make_identity(nc, ident[:])

        # persistent feature tiles
        feat1_f = feats.tile([128, P], F32, tag="feat1_f")
        feat1_b = feats.tile([128, P], F8, tag="feat1_b")
        feat1_a = feats.tile([128, P], F32, tag="feat1_a")  # feat1 + alpha*vb
        feat2_f = feats.tile([128, P], F32, tag="feat2_f")
        feat2_b = feats.tile([128, P], BF16, tag="feat2_b")
        q8 = feats.tile([CQ, P], F8, tag="q8")
        kz = feats.tile([CQ, 3, P], F8, tag="kz")   # [k | 0 | k] pair slots
        vt_all = feats.tile([128, NJC, 130], F8, tag="vt_all")
        f2t_all = feats.tile([128, NJC, 128], BF16, tag="f2t_all")
        sa_feat = feats.tile([128, P], BF16, tag="sa_feat")
        sc_feat = feats.tile([128, P], BF16, tag="sc_feat")
        sa_conv = feats.tile([128, P], BF16, tag="sa_conv")
        sc_conv = feats.tile([128, P], BF16, tag="sc_conv")
        feat_sum = feats.tile([128, P], BF16, tag="feat_sum")

        # ---------------- helpers ----------------
        def conv3_block(psum, rhs2d_list, w_sb, b0, W=512):
            """3-tap conv over output cols [b0, b0+W) into psum [128,W].
            rhs2d_list: list of [128,P] source APs (cin chunks).
            w_sb: [128, 3*nchunks, 128] lhsT per (chunk, tap)."""
            nch = len(rhs2d_list)
            first = True
            for s in (0, -1, 1):
                ol = max(b0, 1) if s == -1 else b0
                oh = min(b0 + W, P - 1) if s == 1 else b0 + W
                for c in range(nch):
                    last = (s == 1 and c == nch - 1)
                    nc.tensor.matmul(
                        psum[:, ol - b0:oh - b0],
                        w_sb[:, c * 3 + (s + 1), :],
                        rhs2d_list[c][:, ol + s:oh + s],
                        start=first, stop=last)
                    first = False

        xs = [x_sb[:, c, :] for c in range(4)]

        # warm the ACT exp table off the critical path (first Exp use
        # triggers a ~2.7us table load)
        warm = smallp.tile([128, 1], F32, tag="warm")
        nc.scalar.activation(warm[:], ident[:, 0:1], AF.Exp)
        nc.gpsimd.memset(kz[:, 1, :].bitcast(mybir.dt.uint32), 0)
        nc.vector.memset(vt_all[:, :, 128:130], 1.0)

        # ---- Phase A: conv5a + qk, interleaved so q/k h0 is ready early ---
        def conv5a_block(b):
            ps = pcc.tile([128, 512], F32, tag="cc")
            conv3_block(ps, xs, w5a, b * 512)
            sl = slice(b * 512, (b + 1) * 512)
            nc.scalar.activation(feat1_f[:, sl], ps[:], AF.Relu, bias=b5a[:])
            nc.gpsimd.tensor_copy(feat1_b[:, sl], feat1_f[:, sl])

        def qk_half(h):
            # fp8 DoubleRow projections: rhs = adjacent 256-col feat1 pairs,
            # lhsT = [w|0] / [0|w] zero-slot variants
            psq = pst.tile([128, 1024], F32, tag="st")
            psk = pst.tile([128, 1024], F32, tag="st")
            for i in range(4):
                c = 4 * h + i   # 256-col chunk index
                if c % 2 == 0:
                    rh = feat1_b[:, c * 256:(c + 2) * 256]
                    s0, s1 = 0, 2
                else:
                    rh = feat1_b[:, (c - 1) * 256:(c + 1) * 256]
                    s0, s1 = 1, 3
                rhp = rh.rearrange("p (two n) -> p two n", two=2)
                hs = slice(i * 256, (i + 1) * 256)
                nc.tensor.matmul(psq[0:CQ, hs], wq3[:, s0:s1, :], rhp,
                                 start=True, stop=True, perf_mode=DR)
                nc.tensor.matmul(psk[0:CQ, hs], wk3[:, s0:s1, :], rhp,
                                 start=True, stop=True, perf_mode=DR)
            for i in range(2):
                sl = slice(h * 1024 + i * 512, h * 1024 + (i + 1) * 512)
                ph = slice(i * 512, (i + 1) * 512)
                nc.vector.tensor_scalar(q8[0:CQ, sl], psq[0:CQ, ph], RSC,
                                        bq[:], op0=OP.mult, op1=OP.add)
                nc.scalar.activation(kz[0:CQ, 0, sl], psk[0:CQ, ph],
                                     AF.Identity, bias=bk[:], scale=RSC)
                nc.gpsimd.tensor_copy(kz[0:CQ, 2, sl], kz[0:CQ, 0, sl])

        conv5a_block(0)
        conv5a_block(1)
        qk_half(0)
        conv5a_block(2)
        conv5a_block(3)
        qk_half(1)
        # feat1 + alpha*vb (for the position-attention residual epilogue)
        nc.vector.tensor_scalar_add(feat1_a[:], feat1_f[:], abpa[:])

        # ---------------- window filler units -----------------------------
        units = []

        def u_vt(g):
            # vT[p,c] = feat1.T @ wv^T via fp8 DR (adjacent p-block pairs)
            def f():
                ps = pcc.tile([128, 512], F32, tag="cc")
                for i in range(4):
                    sub = g * 4 + i
                    if sub % 2 == 0:
                        lh = feat1_b[:, sub * 128:(sub + 2) * 128]
                        s0, s1 = 0, 2
                    else:
                        lh = feat1_b[:, (sub - 1) * 128:(sub + 1) * 128]
                        s0, s1 = 1, 3
                    lhp = lh.rearrange("p (two n) -> p two n", two=2)
                    nc.tensor.matmul(ps[:, i * 128:(i + 1) * 128], lhp,
                                     wv3[:, s0:s1, :],
                                     start=True, stop=True, perf_mode=DR)
                nc.any.tensor_scalar_mul(
                    vt_all[:, g * 4:(g + 1) * 4, 0:128],
                    ps[:].rearrange("p (s c) -> p s c", s=4), RSC)
            return f

        def u_conv5c(hb):
            def f():
                ps = pcc.tile([128, 256], F32, tag="cc")
                conv3_block(ps, xs, w5c, hb * 256, W=256)
                sl = slice(hb * 256, (hb + 1) * 256)
                nc.vector.tensor_scalar(feat2_f[:, sl], ps[:], b5c[:], 0.0,
                                        op0=OP.add, op1=OP.max)
                nc.gpsimd.tensor_copy(feat2_b[:, sl], feat2_f[:, sl])
            return f

        e2_ps = pe2.tile([128, 128], F32, tag="e2")

        def u_f2t(g):
            def f():
                ps = pcc.tile([128, 512], BF16, tag="cc")
                for i in range(4):
                    sub = g * 4 + i
                    nc.tensor.transpose(ps[:, i * 128:(i + 1) * 128],
                                        feat2_b[:, sub * 128:(sub + 1) * 128],
                                        ident[:])
                nc.any.tensor_copy(f2t_all[:, g * 4:(g + 1) * 4, :],
                                   ps[:].rearrange("p (s c) -> p s c", s=4))
                # channel-attention gram accumulation for this group
                for i in range(4):
                    sub = g * 4 + i
                    nc.tensor.matmul(e2_ps[:], f2t_all[:, sub, :],
                                     f2t_all[:, sub, :],
                                     start=(sub == 0), stop=(sub == NJC - 1))
            return f

        attn2 = feats.tile([128, 128], BF16, tag="attn2")
        attn2n = feats.tile([128, 128], BF16, tag="attn2n")
        a2t = feats.tile([128, 128], BF16, tag="a2t")

        def u_softmax2():
            rmin = smallp.tile([128, 1], F32, tag="rmin")
            den2 = smallp.tile([128, 1], F32, tag="den2")
            rden2 = smallp.tile([128, 1], F32, tag="rden2")
            # softmax(max-E) == exp(min-E)/sum: exp(-E + rowmin)
            nc.vector.tensor_reduce(rmin[:], e2_ps[:], axis=AX.X, op=OP.min)
            nc.scalar.activation(attn2[:], e2_ps[:], AF.Exp, bias=rmin[:],
                                 scale=-1.0, accum_out=den2[:])
            nc.vector.reciprocal(rden2[:], den2[:])
            nc.any.tensor_scalar_mul(attn2n[:], attn2[:], rden2[:])
            pt = ptp.tile([128, 128], BF16, tag="tp")
            nc.tensor.transpose(pt[:], attn2n[:], ident[:])
            nc.any.tensor_copy(a2t[:], pt[:])

        def u_out2(b):
            def f():
                ps = pcc.tile([128, 512], F32, tag="cc")
                nc.tensor.matmul(ps[:], a2t[:],
                                 feat2_b[:, b * 512:(b + 1) * 512],
                                 start=True, stop=True)
                # sc_feat = ca_alpha*out2 + feat2
                nc.vector.scalar_tensor_tensor(
                    sc_feat[:, b * 512:(b + 1) * 512], ps[:], alca[:],
                    feat2_f[:, b * 512:(b + 1) * 512], op0=OP.mult, op1=OP.add)
            return f

        def u_c52(b):
            def f():
                ps = pcc.tile([128, 512], F32, tag="cc")
                conv3_block(ps, [sc_feat[:]], w52, b * 512)
                nc.vector.tensor_scalar(sc_conv[:, b * 512:(b + 1) * 512],
                                        ps[:], b52[:], 0.0,
                                        op0=OP.add, op1=OP.max)
            return f

        def u_c51w(o0):
            # in-window c51 block: psum from cc, relu+add on DVE (ACT is the
            # window bottleneck); needs sa_feat cols <= o0+512+1
            def f():
                sl = slice(o0, o0 + 512)
                ps = pcc.tile([128, 512], F32, tag="cc")
                conv3_block(ps, [sa_feat[:]], w51, o0)
                nc.vector.tensor_scalar(sa_conv[:, sl], ps[:], b51[:], 0.0,
                                        op0=OP.add, op1=OP.max)
                nc.vector.tensor_add(feat_sum[:, sl], sa_conv[:, sl],
                                     sc_conv[:, sl])
            return f

        def u_c8w(o0, co):
            def f():
                sl = slice(o0, o0 + 512)
                p8 = pcc.tile([128, 512], F32, tag="cc")
                nc.tensor.matmul(p8[:], w8[:, co, :], feat_sum[:, sl],
                                 start=True, stop=True)
                ot = outp.tile([128, 512], BF16, tag="out_sb", bufs=6)
                nc.vector.tensor_scalar_add(ot[:], p8[:], b8[:, co:co + 1])
                nc.sync.dma_start(dout[co, :, sl], ot[:])
            return f

        for hb in range(8):
            units.append((u_conv5c(hb), 800))
            if hb < 4:
                units.append((u_vt(hb), 600))
        for g in range(4):
            units.append((u_f2t(g), 600))
        units.append((u_softmax2, 300))
        for b in range(4):
            units.append((u_out2(b), 250))
        for b in range(4):
            units.append((u_c52(b), 700))
        units.append((u_c51w(0), 1000))
        for co in range(4):
            units.append((u_c8w(0, co), 600))
        units.append((u_c51w(512), 1000))
        for co in range(4):
            units.append((u_c8w(512, co), 600))

        # ---------------- AV emitter (used in window + after) -------------
        def emit_av(isub):
            ps = pcc.tile([128, 132], F32, tag="cc")
            off = (isub % 4) * 128
            for jcp in range(8):
                est = es2[jcp * 4 + isub // 4]
                lh = est[:].rearrange("p (two n) -> p two n", two=2)
                nc.tensor.matmul(ps[:, 0:129], lh[:, :, off:off + 128],
                                 vt_all[:, 2 * jcp:2 * jcp + 2, 0:129],
                                 start=(jcp == 0), stop=(jcp == 7),
                                 perf_mode=DR)
            rcol = smallp.tile([128, 1], F32, tag="rcol", bufs=8)
            nc.vector.reciprocal(rcol[:], ps[:, 128:129])
            onrm = smallp.tile([128, 128], BF16, tag="onrm", bufs=4)
            nc.any.tensor_scalar_mul(onrm[:], ps[:, 0:128], rcol[:])
            tpool = ptp if isub % 2 == 0 else pe2
            ttag = "tp" if isub % 2 == 0 else "e2"
            pt = tpool.tile([128, 128], BF16, tag=ttag)
            nc.tensor.transpose(pt[:], onrm[:], ident[:])
            # sa_feat = alpha*outT + (feat1 + alpha*vb)
            nc.vector.scalar_tensor_tensor(
                sa_feat[:, isub * 128:(isub + 1) * 128], pt[:], alpa[:],
                feat1_a[:, isub * 128:(isub + 1) * 128],
                op0=OP.mult, op1=OP.add)

        # ---------------- Phase B: S^T + exp window -----------------------
        # S_T[j, i] = sum_d k[d,j] q[d,i]; exp -> expS (bf16).
        # 2-way row-tiled: strips (0,0)/(64,0) compute jc pair (2t, 2t+1)
        # concurrently. i-block-major order so AV isubs start mid-window.
        # es2[t*4+b]: [128, 0:512]=expS[2t][:, b*512:], [512:]=expS[2t+1].
        es2 = [None] * 32
        # step order: first 8 steps need only the h0 halves of q/k (pairs
        # t<4 x columns b<2), so exps start while conv5a b2/b3 + qk h1 are
        # still in flight; then b-major for the rest.
        order = [(t, b) for t in range(4) for b in range(2)]
        seen = set(order)
        for b in range(4):
            for t in range(8):
                if (t, b) not in seen:
                    order.append((t, b))
        colcnt = [0] * 4
        av_next = 0
        for (t, b) in order:
            es = expsp.tile([128, 1024], F8, tag="expS",
                            name=f"es{t}_{b}")
            es2[t * 4 + b] = es
            ps = pst.tile([128, 1024], F32, tag="st")
            qp = q8[0:CQ, b * 512:(b + 1) * 512].rearrange(
                "k (two n) -> k two n", two=2)
            for j in range(2):
                jc = 2 * t + j
                jb = slice(jc * 128, (jc + 1) * 128)
                o = j * 512
                nc.tensor.matmul(ps[:, o:o + 256], kz[:, 0:2, jb], qp,
                                 start=True, stop=True, perf_mode=DR)
                nc.tensor.matmul(ps[:, o + 256:o + 512], kz[:, 1:3, jb], qp,
                                 start=True, stop=True, perf_mode=DR)
            nc.scalar.activation(es[:], ps[:], AF.Exp)
            colcnt[b] += 1
            # keep the PE just behind the ACT exp rate (~1us/step)
            budget = 300.0
            while units and budget > 0:
                f, cost = units.pop(0)
                f()
                budget -= cost
            # AV isubs for completed i-columns ride inside the window
            if av_next < 12 and colcnt[av_next // 4] == 8:
                emit_av(av_next)
                av_next += 1
        # avs 12/13 first: their early MMs depend on es2 tiles finished
        # several window steps ago, so they overlap the window tail; the
        # few leftover units follow in the stream.
        emit_av(12)
        emit_av(13)
        while units:
            units.pop(0)[0]()

        # ------- Phase C/D: AV isubs 8..15 + tail woven in ----------------
        def t_conv(o0, W=512):
            """c51 cols [o0, o0+W) -> feat_sum (ACT relu: ACT is idle here)."""
            sl = slice(o0, o0 + W)
            ps = pst.tile([128, 512], F32, tag="st")
            conv3_block(ps[:, 0:W], [sa_feat[:]], w51, o0, W=W)
            nc.scalar.activation(sa_conv[:, sl], ps[:, 0:W], AF.Relu,
                                 bias=b51[:])
            nc.vector.tensor_add(feat_sum[:, sl], sa_conv[:, sl],
                                 sc_conv[:, sl])

        def t_c8(o0, co, W=512):
            sl = slice(o0, o0 + W)
            p8 = pst.tile([128, 512], F32, tag="st")
            nc.tensor.matmul(p8[:, 0:W], w8[:, co, :], feat_sum[:, sl],
                             start=True, stop=True)
            ot = outp.tile([128, 512], BF16, tag="out_sb", bufs=6)
            nc.any.tensor_scalar_add(ot[:, 0:W], p8[:, 0:W], b8[:, co:co + 1])
            nc.sync.dma_start(dout[co, :, sl], ot[:, 0:W])

        # c51 cols [o, o+W) need sa_feat cols <= o+W, i.e. isubs <= (o+W)/128
        # (isubs 0..11 completed inside the window)
        t_conv(1024)
        emit_av(14)
        t_c8(1024, 0)
        emit_av(15)
        t_c8(1024, 1)
        t_c8(1024, 2)
        t_c8(1024, 3)
        # final c51 block in engine-parallel halves: ACT does one relu while
        # DVE does the other; adds on DVE/gpsimd — shortens the last chain
        slA = slice(1536, 1792)
        psA = pst.tile([128, 512], F32, tag="st", name="c51fA")
        conv3_block(psA[:, 0:256], [sa_feat[:]], w51, 1536, W=256)
        nc.scalar.activation(sa_conv[:, slA], psA[:, 0:256], AF.Relu,
                             bias=b51[:])
        nc.vector.tensor_add(feat_sum[:, slA], sa_conv[:, slA],
                             sc_conv[:, slA])
        slB = slice(1792, 2048)
        psB = pcc.tile([128, 512], F32, tag="cc", name="c51fB")
        conv3_block(psB[:, 0:256], [sa_feat[:]], w51, 1792, W=256)
        nc.vector.tensor_scalar(sa_conv[:, slB], psB[:, 0:256], b51[:], 0.0,
                                op0=OP.add, op1=OP.max)
        nc.vector.tensor_add(feat_sum[:, slB], sa_conv[:, slB],
                             sc_conv[:, slB])
        t_c8(1536, 0)
        t_c8(1536, 1)
        t_c8(1536, 2)
        t_c8(1536, 3)

    nc.compile()
    return nc


_NC = None


def _get_nc():
    global _NC
    if _NC is None:
        _NC = _build_module()
    return _NC


def _zslot(w):  # [128, C] f32 -> [128, 3, C] fp8 = [w*SC | 0 | w*SC]
    z = np.zeros((128, 3, w.shape[1]), NPF8)
    ws = (w * SC).astype(NPF8)
    z[:, 0, :] = ws
    z[:, 2, :] = ws
    return z


def _prep_inputs(inputs):
    """Host-side: fold BN into conv weights, transpose to lhsT layouts,
    cast matmul operands to bf16. Returns (shared_map, per_core_x)."""
    f32 = np.float32

    def fold(w, g, b, m, v):
        s = (g / np.sqrt(v + EPS)).astype(f32)
        return (w * s[:, None, None]).astype(f32), (b - m * s).astype(f32)

    w5a, b5a = fold(inputs['c5a_w'], inputs['c5a_g'], inputs['c5a_b'],
                    inputs['c5a_m'], inputs['c5a_v'])
    w5c, b5c = fold(inputs['c5c_w'], inputs['c5c_g'], inputs['c5c_b'],
                    inputs['c5c_m'], inputs['c5c_v'])
    w51, b51 = fold(inputs['c51_w'], inputs['c51_g'], inputs['c51_b'],
                    inputs['c51_m'], inputs['c51_v'])
    w52, b52 = fold(inputs['c52_w'], inputs['c52_g'], inputs['c52_b'],
                    inputs['c52_m'], inputs['c52_v'])

    def big_lhsT(w):  # [128, 512, 3] -> [p, chunk*3+tap, c] = [128, 12, 128]
        return np.ascontiguousarray(
            w.reshape(128, 4, 128, 3).transpose(2, 1, 3, 0)
        ).reshape(128, 12, 128)

    def small_lhsT(w):  # [128, 128, 3] -> [p, tap, c] = [128, 3, 128]
        return np.ascontiguousarray(w.transpose(1, 2, 0))

    pa = float(np.asarray(inputs['pa_alpha']).reshape(-1)[0])
    ca = float(np.asarray(inputs['ca_alpha']).reshape(-1)[0])

    shared = {
        'w5a': big_lhsT(w5a).astype(NPBF),
        'b5a': b5a.reshape(128, 1),
        'w5c': big_lhsT(w5c).astype(NPBF),
        'b5c': b5c.reshape(128, 1),
        'wq3': _zslot(inputs['qw'][:, :, 0].T.astype(f32)),
        'wk3': _zslot(inputs['kw'][:, :, 0].T.astype(f32)),
        'bq': np.asarray(inputs['qb']).reshape(CQ, 1).astype(f32),
        'bk': np.asarray(inputs['kb']).reshape(CQ, 1).astype(f32),
        'wv3': _zslot(inputs['vw'][:, :, 0].T.astype(f32)),
        'w51': small_lhsT(w51).astype(NPBF),
        'b51': b51.reshape(128, 1),
        'w52': small_lhsT(w52).astype(NPBF),
        'b52': b52.reshape(128, 1),
        'w8': np.ascontiguousarray(
            inputs['c8_w'][:, :, 0].reshape(4, 128, 128).transpose(2, 0, 1)
        ).astype(NPBF),
        'b8': np.ascontiguousarray(
            inputs['c8_b'].reshape(4, 128).T).astype(f32),
        'alpa': np.full((128, 1), pa, f32),
        'abpa': (pa * np.asarray(inputs['vb'])).reshape(128, 1).astype(f32),
        'alca': np.full((128, 1), ca, f32),
    }
    shared = {k: np.ascontiguousarray(v) for k, v in shared.items()}

    x = np.asarray(inputs['x'])  # [8, 512, 2048]
    per_core_x = [
        np.ascontiguousarray(
            x[b].reshape(4, 128, P).transpose(1, 0, 2).astype(NPBF))
        for b in range(NCORES)
    ]
    return shared, per_core_x


def kernel(**inputs) -> np.ndarray:
    inputs = {k: np.asarray(v) for k, v in inputs.items()}
    nc = _get_nc()
    shared, per_core_x = _prep_inputs(inputs)
    in_maps = [dict(shared, x=per_core_x[b]) for b in range(NCORES)]
    last_err = None
    for _attempt in range(3):
        try:
            res = run_bass_kernel_spmd(nc, in_maps,
                                       core_ids=list(range(NCORES)))
            break
        except Exception as e:  # transient device errors: retry
            last_err = e
            import time as _time
            _time.sleep(2.0)
    else:
        raise last_err
    out = np.stack([res.results[b]['out'].reshape(COUT, P)
                    for b in range(NCORES)])
    return out.astype(np.float32)



# revision 34
# speedup vs baseline: 1.1338x; 1.0055x over previous
"""DualAttention (position attention + channel attention) Trainium2 kernel.

Data-parallel over batch: 8 samples -> 8 NeuronCores, weights replicated.
All heavy matmuls run in bf16 (f32 PSUM accumulation); softmax math,
residual adds and the final output stay f32.

Self-contained: shapes/sharding hardcoded, no sibling imports.
"""

import numpy as np
import ml_dtypes
from contextlib import ExitStack

import concourse.bass as bass
import concourse.tile as tile
from concourse import bacc, mybir
from concourse.bass_utils import run_bass_kernel_spmd
from concourse.masks import make_identity

F32 = mybir.dt.float32
BF16 = mybir.dt.bfloat16
F8 = mybir.dt.float8e4
DR = mybir.MatmulPerfMode.DoubleRow
AF = mybir.ActivationFunctionType
OP = mybir.AluOpType
AX = mybir.AxisListType
NPBF = ml_dtypes.bfloat16
NPF8 = ml_dtypes.float8_e4m3

EPS = 1e-5
SC = 16.0     # fp8 weight scale (keeps w*16 in e4m3 normal range)
RSC = 1.0 / SC
P = 2048      # positions
CIN = 512     # input channels (4 chunks of 128)
CI = 128      # inner channels
CQ = 16       # q/k channels
COUT = 512    # output channels (4 chunks of 128)
NCORES = 8
NJC = P // 128   # 16 j-chunks / p-subtiles


def _build_module():
    nc = bacc.Bacc("TRN2", target_bir_lowering=False, debug=False,
                   num_devices=NCORES)

    # ---------------- DRAM I/O ----------------
    dx = nc.dram_tensor("x", [128, 4, P], BF16, kind="ExternalInput")
    dw5a = nc.dram_tensor("w5a", [128, 12, 128], BF16, kind="ExternalInput")
    db5a = nc.dram_tensor("b5a", [128, 1], F32, kind="ExternalInput")
    dw5c = nc.dram_tensor("w5c", [128, 12, 128], BF16, kind="ExternalInput")
    db5c = nc.dram_tensor("b5c", [128, 1], F32, kind="ExternalInput")
    dwq3 = nc.dram_tensor("wq3", [128, 3, CQ], F8, kind="ExternalInput")
    dwk3 = nc.dram_tensor("wk3", [128, 3, CQ], F8, kind="ExternalInput")
    dbq = nc.dram_tensor("bq", [CQ, 1], F32, kind="ExternalInput")
    dbk = nc.dram_tensor("bk", [CQ, 1], F32, kind="ExternalInput")
    dwv3 = nc.dram_tensor("wv3", [128, 3, 128], F8, kind="ExternalInput")
    dw51 = nc.dram_tensor("w51", [128, 3, 128], BF16, kind="ExternalInput")
    db51 = nc.dram_tensor("b51", [128, 1], F32, kind="ExternalInput")
    dw52 = nc.dram_tensor("w52", [128, 3, 128], BF16, kind="ExternalInput")
    db52 = nc.dram_tensor("b52", [128, 1], F32, kind="ExternalInput")
    dw8 = nc.dram_tensor("w8", [128, 4, 128], BF16, kind="ExternalInput")
    db8 = nc.dram_tensor("b8", [128, 4], F32, kind="ExternalInput")
    dalpa = nc.dram_tensor("alpa", [128, 1], F32, kind="ExternalInput")
    dabpa = nc.dram_tensor("abpa", [128, 1], F32, kind="ExternalInput")
    dalca = nc.dram_tensor("alca", [128, 1], F32, kind="ExternalInput")
    dout = nc.dram_tensor("out", [4, 128, P], BF16, kind="ExternalOutput")

    with tile.TileContext(nc) as tc, ExitStack() as ctx:
        const = ctx.enter_context(tc.tile_pool(name="const", bufs=1))
        feats = ctx.enter_context(tc.tile_pool(name="feats", bufs=1))
        expsp = ctx.enter_context(tc.tile_pool(name="expsp", bufs=32))
        outp = ctx.enter_context(tc.tile_pool(name="outp", bufs=2))
        smallp = ctx.enter_context(tc.tile_pool(name="smallp", bufs=4))
        # PSUM: st 2x[128,2048]bf16 (4 banks) + cc 2x[128,512]f32 (2 banks)
        #       + tp 1x[128,128]f32 (1 bank) + e2 1x[128,128]f32 (1 bank)
        pst = ctx.enter_context(tc.tile_pool(name="pst", bufs=2, space="PSUM"))
        pcc = ctx.enter_context(tc.tile_pool(name="pcc", bufs=2, space="PSUM"))
        ptp = ctx.enter_context(tc.tile_pool(name="ptp", bufs=1, space="PSUM"))
        pe2 = ctx.enter_context(tc.tile_pool(name="pe2", bufs=1, space="PSUM"))

        # ---------------- constants in ----------------
        _dma_rr = [nc.sync, nc.sync]
        _dma_i = [0]

        def cload(name, shape, dtype, dram):
            t = const.tile(shape, dtype, tag=name)
            eng = _dma_rr[_dma_i[0] % len(_dma_rr)]
            _dma_i[0] += 1
            eng.dma_start(t[:], dram[:])
            return t

        w5a = const.tile([128, 12, 128], BF16, tag="w5a")
        x_sb = const.tile([128, 4, P], BF16, tag="x")
        # DMA dispatch costs ~0.65us of sequencer time each; spread the head
        # transfers across otherwise-idle sequencers so the first conv
        # operands land as early as possible.
        nc.sync.dma_start(w5a[:, 0:6, :], dw5a[:, 0:6, :])
        nc.gpsimd.dma_start(x_sb[:, 0, 0:516], dx[:, 0, 0:516])
        nc.gpsimd.dma_start(x_sb[:, 1, 0:516], dx[:, 1, 0:516])
        nc.sync.dma_start(w5a[:, 6:12, :], dw5a[:, 6:12, :])
        nc.sync.dma_start(x_sb[:, 2, 0:516], dx[:, 2, 0:516])
        nc.sync.dma_start(x_sb[:, 3, 0:516], dx[:, 3, 0:516])
        b5a = cload("b5a", [128, 1], F32, db5a)
        xsplit = [516, 1028, 1540, 2048]
        xeng = [nc.sync, nc.sync, nc.sync]
        for r in range(3):
            xeng[r].dma_start(x_sb[:, :, xsplit[r]:xsplit[r + 1]],
                              dx[:, :, xsplit[r]:xsplit[r + 1]])
        wq3 = cload("wq3", [128, 3, CQ], F8, dwq3)
        wk3 = cload("wk3", [128, 3, CQ], F8, dwk3)
        bq = cload("bq", [CQ, 1], F32, dbq)
        bk = cload("bk", [CQ, 1], F32, dbk)
        wv3 = cload("wv3", [128, 3, 128], F8, dwv3)
        abpa = cload("abpa", [128, 1], F32, dabpa)
        w5c = cload("w5c", [128, 12, 128], BF16, dw5c)
        b5c = cload("b5c", [128, 1], F32, db5c)
        w51 = cload("w51", [128, 3, 128], BF16, dw51)
        b51 = cload("b51", [128, 1], F32, db51)
        w52 = cload("w52", [128, 3, 128], BF16, dw52)
        b52 = cload("b52", [128, 1], F32, db52)
        w8 = cload("w8", [128, 4, 128], BF16, dw8)
        b8 = cload("b8", [128, 4], F32, db8)
        alpa = cload("alpa", [128, 1], F32, dalpa)
        alca = cload("alca", [128, 1], F32, dalca)

        ident = const.tile([128, 128], BF16, tag="ident")
        make_identity(nc, ident[:])

        # persistent feature tiles
        feat1_f = feats.tile([128, P], F32, tag="feat1_f")
        feat1_b = feats.tile([128, P], F8, tag="feat1_b")
        feat1_a = feats.tile([128, P], F32, tag="feat1_a")  # feat1 + alpha*vb
        feat2_f = feats.tile([128, P], F32, tag="feat2_f")
        feat2_b = feats.tile([128, P], BF16, tag="feat2_b")
        q8 = feats.tile([CQ, P], F8, tag="q8")
        kz = feats.tile([CQ, 3, P], F8, tag="kz")   # [k | 0 | k] pair slots
        vt_all = feats.tile([128, NJC, 130], F8, tag="vt_all")
        f2t_all = feats.tile([128, NJC, 128], BF16, tag="f2t_all")
        sa_feat = feats.tile([128, P], BF16, tag="sa_feat")
        sc_feat = feats.tile([128, P], BF16, tag="sc_feat")
        sa_conv = feats.tile([128, P], BF16, tag="sa_conv")
        sc_conv = feats.tile([128, P], BF16, tag="sc_conv")
        feat_sum = feats.tile([128, P], BF16, tag="feat_sum")

        # ---------------- helpers ----------------
        def conv3_block(psum, rhs2d_list, w_sb, b0, W=512):
            """3-tap conv over output cols [b0, b0+W) into psum [128,W].
            rhs2d_list: list of [128,P] source APs (cin chunks).
            w_sb: [128, 3*nchunks, 128] lhsT per (chunk, tap)."""
            nch = len(rhs2d_list)
            first = True
            for s in (0, -1, 1):
                ol = max(b0, 1) if s == -1 else b0
                oh = min(b0 + W, P - 1) if s == 1 else b0 + W
                for c in range(nch):
                    last = (s == 1 and c == nch - 1)
                    nc.tensor.matmul(
                        psum[:, ol - b0:oh - b0],
                        w_sb[:, c * 3 + (s + 1), :],
                        rhs2d_list[c][:, ol + s:oh + s],
                        start=first, stop=last)
                    first = False

        xs = [x_sb[:, c, :] for c in range(4)]

        # warm the ACT exp table off the critical path (first Exp use
        # triggers a ~2.7us table load)
        warm = smallp.tile([128, 1], F32, tag="warm")
        nc.scalar.activation(warm[:], ident[:, 0:1], AF.Exp)
        nc.gpsimd.memset(kz[:, 1, :].bitcast(mybir.dt.uint32), 0)
        nc.vector.memset(vt_all[:, :, 128:130], 1.0)

        # ---- Phase A: conv5a + qk, interleaved so q/k h0 is ready early ---
        def conv5a_block(b):
            ps = pcc.tile([128, 512], F32, tag="cc")
            conv3_block(ps, xs, w5a, b * 512)
            sl = slice(b * 512, (b + 1) * 512)
            nc.scalar.activation(feat1_f[:, sl], ps[:], AF.Relu, bias=b5a[:])
            nc.gpsimd.tensor_copy(feat1_b[:, sl], feat1_f[:, sl])

        def qk_half(h):
            # fp8 DoubleRow projections: rhs = adjacent 256-col feat1 pairs,
            # lhsT = [w|0] / [0|w] zero-slot variants
            psq = pst.tile([128, 1024], F32, tag="st")
            psk = pst.tile([128, 1024], F32, tag="st")
            for i in range(4):
                c = 4 * h + i   # 256-col chunk index
                if c % 2 == 0:
                    rh = feat1_b[:, c * 256:(c + 2) * 256]
                    s0, s1 = 0, 2
                else:
                    rh = feat1_b[:, (c - 1) * 256:(c + 1) * 256]
                    s0, s1 = 1, 3
                rhp = rh.rearrange("p (two n) -> p two n", two=2)
                hs = slice(i * 256, (i + 1) * 256)
                nc.tensor.matmul(psq[0:CQ, hs], wq3[:, s0:s1, :], rhp,
                                 start=True, stop=True, perf_mode=DR)
                nc.tensor.matmul(psk[0:CQ, hs], wk3[:, s0:s1, :], rhp,
                                 start=True, stop=True, perf_mode=DR)
            for i in range(2):
                sl = slice(h * 1024 + i * 512, h * 1024 + (i + 1) * 512)
                ph = slice(i * 512, (i + 1) * 512)
                nc.vector.tensor_scalar(q8[0:CQ, sl], psq[0:CQ, ph], RSC,
                                        bq[:], op0=OP.mult, op1=OP.add)
                nc.scalar.activation(kz[0:CQ, 0, sl], psk[0:CQ, ph],
                                     AF.Identity, bias=bk[:], scale=RSC)
                nc.gpsimd.tensor_copy(kz[0:CQ, 2, sl], kz[0:CQ, 0, sl])

        es2 = [None] * 32

        def st_step(t, b):
            es = expsp.tile([128, 1024], F8, tag="expS",
                            name=f"es{t}_{b}")
            es2[t * 4 + b] = es
            ps = pst.tile([128, 1024], F32, tag="st")
            qp = q8[0:CQ, b * 512:(b + 1) * 512].rearrange(
                "k (two n) -> k two n", two=2)
            for j in range(2):
                jc = 2 * t + j
                jb = slice(jc * 128, (jc + 1) * 128)
                o = j * 512
                nc.tensor.matmul(ps[:, o:o + 256], kz[:, 0:2, jb], qp,
                                 start=True, stop=True, perf_mode=DR)
                nc.tensor.matmul(ps[:, o + 256:o + 512], kz[:, 1:3, jb], qp,
                                 start=True, stop=True, perf_mode=DR)
            nc.scalar.activation(es[:], ps[:], AF.Exp)

        conv5a_block(0)
        conv5a_block(1)
        qk_half(0)
        # first two window steps need only q/k cols [0:1024) = qk_half(0):
        # start the exp pipeline while conv5a b2/b3 + qk h1 are in flight
        st_step(0, 0)
        st_step(0, 1)
        conv5a_block(2)
        conv5a_block(3)
        qk_half(1)
        # feat1 + alpha*vb (for the position-attention residual epilogue)
        nc.vector.tensor_scalar_add(feat1_a[:], feat1_f[:], abpa[:])

        # ---------------- window filler units -----------------------------
        units = []

        def u_vt(g):
            # vT[p,c] = feat1.T @ wv^T via fp8 DR (adjacent p-block pairs)
            def f():
                ps = pcc.tile([128, 512], F32, tag="cc")
                for i in range(4):
                    sub = g * 4 + i
                    if sub % 2 == 0:
                        lh = feat1_b[:, sub * 128:(sub + 2) * 128]
                        s0, s1 = 0, 2
                    else:
                        lh = feat1_b[:, (sub - 1) * 128:(sub + 1) * 128]
                        s0, s1 = 1, 3
                    lhp = lh.rearrange("p (two n) -> p two n", two=2)
                    nc.tensor.matmul(ps[:, i * 128:(i + 1) * 128], lhp,
                                     wv3[:, s0:s1, :],
                                     start=True, stop=True, perf_mode=DR)
                nc.any.tensor_scalar_mul(
                    vt_all[:, g * 4:(g + 1) * 4, 0:128],
                    ps[:].rearrange("p (s c) -> p s c", s=4), RSC)
            return f

        def u_conv5c(hb):
            def f():
                ps = pcc.tile([128, 256], F32, tag="cc")
                conv3_block(ps, xs, w5c, hb * 256, W=256)
                sl = slice(hb * 256, (hb + 1) * 256)
                nc.vector.tensor_scalar(feat2_f[:, sl], ps[:], b5c[:], 0.0,
                                        op0=OP.add, op1=OP.max)
                nc.gpsimd.tensor_copy(feat2_b[:, sl], feat2_f[:, sl])
            return f

        e2_ps = pe2.tile([128, 128], F32, tag="e2")

        def u_f2t(g):
            def f():
                ps = pcc.tile([128, 512], BF16, tag="cc")
                for i in range(4):
                    sub = g * 4 + i
                    nc.tensor.transpose(ps[:, i * 128:(i + 1) * 128],
                                        feat2_b[:, sub * 128:(sub + 1) * 128],
                                        ident[:])
                nc.any.tensor_copy(f2t_all[:, g * 4:(g + 1) * 4, :],
                                   ps[:].rearrange("p (s c) -> p s c", s=4))
                # channel-attention gram accumulation for this group
                for i in range(4):
                    sub = g * 4 + i
                    nc.tensor.matmul(e2_ps[:], f2t_all[:, sub, :],
                                     f2t_all[:, sub, :],
                                     start=(sub == 0), stop=(sub == NJC - 1))
            return f

        attn2 = feats.tile([128, 128], BF16, tag="attn2")
        attn2n = feats.tile([128, 128], BF16, tag="attn2n")
        a2t = feats.tile([128, 128], BF16, tag="a2t")

        def u_softmax2():
            rmin = smallp.tile([128, 1], F32, tag="rmin")
            den2 = smallp.tile([128, 1], F32, tag="den2")
            rden2 = smallp.tile([128, 1], F32, tag="rden2")
            # softmax(max-E) == exp(min-E)/sum: exp(-E + rowmin)
            nc.vector.tensor_reduce(rmin[:], e2_ps[:], axis=AX.X, op=OP.min)
            nc.scalar.activation(attn2[:], e2_ps[:], AF.Exp, bias=rmin[:],
                                 scale=-1.0, accum_out=den2[:])
            nc.vector.reciprocal(rden2[:], den2[:])
            nc.any.tensor_scalar_mul(attn2n[:], attn2[:], rden2[:])
            pt = ptp.tile([128, 128], BF16, tag="tp")
            nc.tensor.transpose(pt[:], attn2n[:], ident[:])
            nc.any.tensor_copy(a2t[:], pt[:])

        def u_out2(b):
            def f():
                ps = pcc.tile([128, 512], F32, tag="cc")
                nc.tensor.matmul(ps[:], a2t[:],
                                 feat2_b[:, b * 512:(b + 1) * 512],
                                 start=True, stop=True)
                # sc_feat = ca_alpha*out2 + feat2
                nc.vector.scalar_tensor_tensor(
                    sc_feat[:, b * 512:(b + 1) * 512], ps[:], alca[:],
                    feat2_f[:, b * 512:(b + 1) * 512], op0=OP.mult, op1=OP.add)
            return f

        def u_c52(b):
            def f():
                ps = pcc.tile([128, 512], F32, tag="cc")
                conv3_block(ps, [sc_feat[:]], w52, b * 512)
                nc.vector.tensor_scalar(sc_conv[:, b * 512:(b + 1) * 512],
                                        ps[:], b52[:], 0.0,
                                        op0=OP.add, op1=OP.max)
            return f

        def u_c51w(o0):
            # in-window c51 block: psum from cc, relu+add on DVE (ACT is the
            # window bottleneck); needs sa_feat cols <= o0+512+1
            def f():
                sl = slice(o0, o0 + 512)
                ps = pcc.tile([128, 512], F32, tag="cc")
                conv3_block(ps, [sa_feat[:]], w51, o0)
                nc.vector.tensor_scalar(sa_conv[:, sl], ps[:], b51[:], 0.0,
                                        op0=OP.add, op1=OP.max)
                nc.vector.tensor_add(feat_sum[:, sl], sa_conv[:, sl],
                                     sc_conv[:, sl])
            return f

        def u_c8w(o0, co):
            def f():
                sl = slice(o0, o0 + 512)
                p8 = pcc.tile([128, 512], F32, tag="cc")
                nc.tensor.matmul(p8[:], w8[:, co, :], feat_sum[:, sl],
                                 start=True, stop=True)
                ot = outp.tile([128, 512], BF16, tag="out_sb", bufs=6)
                nc.vector.tensor_scalar_add(ot[:], p8[:], b8[:, co:co + 1])
                nc.sync.dma_start(dout[co, :, sl], ot[:])
            return f

        for hb in range(8):
            units.append((u_conv5c(hb), 800))
            if hb < 4:
                units.append((u_vt(hb), 600))
        for g in range(4):
            units.append((u_f2t(g), 600))
        units.append((u_softmax2, 300))
        for b in range(4):
            units.append((u_out2(b), 250))
        for b in range(4):
            units.append((u_c52(b), 700))
        units.append((u_c51w(0), 1000))
        for co in range(4):
            units.append((u_c8w(0, co), 600))
        units.append((u_c51w(512), 1000))
        for co in range(4):
            units.append((u_c8w(512, co), 600))

        # ---------------- AV emitter (used in window + after) -------------
        def emit_av(isub):
            ps = pcc.tile([128, 132], F32, tag="cc")
            off = (isub % 4) * 128
            for jcp in range(8):
                est = es2[jcp * 4 + isub // 4]
                lh = est[:].rearrange("p (two n) -> p two n", two=2)
                nc.tensor.matmul(ps[:, 0:129], lh[:, :, off:off + 128],
                                 vt_all[:, 2 * jcp:2 * jcp + 2, 0:129],
                                 start=(jcp == 0), stop=(jcp == 7),
                                 perf_mode=DR)
            rcol = smallp.tile([128, 1], F32, tag="rcol", bufs=8)
            nc.vector.reciprocal(rcol[:], ps[:, 128:129])
            onrm = smallp.tile([128, 128], BF16, tag="onrm", bufs=4)
            nc.any.tensor_scalar_mul(onrm[:], ps[:, 0:128], rcol[:])
            tpool = ptp if isub % 2 == 0 else pe2
            ttag = "tp" if isub % 2 == 0 else "e2"
            pt = tpool.tile([128, 128], BF16, tag=ttag)
            nc.tensor.transpose(pt[:], onrm[:], ident[:])
            # sa_feat = alpha*outT + (feat1 + alpha*vb)
            nc.vector.scalar_tensor_tensor(
                sa_feat[:, isub * 128:(isub + 1) * 128], pt[:], alpa[:],
                feat1_a[:, isub * 128:(isub + 1) * 128],
                op0=OP.mult, op1=OP.add)

        # ---------------- Phase B: S^T + exp window -----------------------
        # S_T[j, i] = sum_d k[d,j] q[d,i]; exp -> expS (fp8, DoubleRow).
        # es2[t*4+b]: [128, 0:512]=expS[2t][:, b*512:], [512:]=expS[2t+1].
        order = [(t, b) for t in range(4) for b in range(2)
                 if (t, b) not in ((0, 0), (0, 1))]
        seen = set(order) | {(0, 0), (0, 1)}
        for b in range(4):
            for t in range(8):
                if (t, b) not in seen:
                    order.append((t, b))
        colcnt = [1, 1, 0, 0]   # (0,0)/(0,1) woven into the head
        av_next = 0
        for (t, b) in order:
            st_step(t, b)
            colcnt[b] += 1
            # keep the PE just behind the ACT exp rate (~1us/step)
            budget = 300.0
            while units and budget > 0:
                f, cost = units.pop(0)
                f()
                budget -= cost
            # AV isubs for completed i-columns ride inside the window
            if av_next < 12 and colcnt[av_next // 4] == 8:
                emit_av(av_next)
                av_next += 1
        # avs 12/13 first: their early MMs depend on es2 tiles finished
        # several window steps ago, so they overlap the window tail; the
        # few leftover units follow in the stream.
        emit_av(12)
        emit_av(13)
        while units:
            units.pop(0)[0]()

        # ------- Phase C/D: AV isubs 8..15 + tail woven in ----------------
        def t_conv(o0, W=512):
            """c51 cols [o0, o0+W) -> feat_sum (ACT relu: ACT is idle here)."""
            sl = slice(o0, o0 + W)
            ps = pst.tile([128, 512], F32, tag="st")
            conv3_block(ps[:, 0:W], [sa_feat[:]], w51, o0, W=W)
            nc.scalar.activation(sa_conv[:, sl], ps[:, 0:W], AF.Relu,
                                 bias=b51[:])
            nc.vector.tensor_add(feat_sum[:, sl], sa_conv[:, sl],
                                 sc_conv[:, sl])

        def t_c8(o0, co, W=512):
            sl = slice(o0, o0 + W)
            p8 = pst.tile([128, 512], F32, tag="st")
            nc.tensor.matmul(p8[:, 0:W], w8[:, co, :], feat_sum[:, sl],
                             start=True, stop=True)
            ot = outp.tile([128, 512], BF16, tag="out_sb", bufs=6)
            nc.any.tensor_scalar_add(ot[:, 0:W], p8[:, 0:W], b8[:, co:co + 1])
            nc.sync.dma_start(dout[co, :, sl], ot[:, 0:W])

        # c51 cols [o, o+W) need sa_feat cols <= o+W, i.e. isubs <= (o+W)/128
        # (isubs 0..11 completed inside the window)
        t_conv(1024)
        emit_av(14)
        t_c8(1024, 0)
        emit_av(15)
        t_c8(1024, 1)
        t_c8(1024, 2)
        t_c8(1024, 3)
        # final c51 block in engine-parallel halves: ACT does one relu while
        # DVE does the other; adds on DVE/gpsimd — shortens the last chain
        slA = slice(1536, 1792)
        psA = pst.tile([128, 512], F32, tag="st", name="c51fA")
        conv3_block(psA[:, 0:256], [sa_feat[:]], w51, 1536, W=256)
        nc.scalar.activation(sa_conv[:, slA], psA[:, 0:256], AF.Relu,
                             bias=b51[:])
        nc.vector.tensor_add(feat_sum[:, slA], sa_conv[:, slA],
                             sc_conv[:, slA])
        slB = slice(1792, 2048)
        psB = pcc.tile([128, 512], F32, tag="cc", name="c51fB")
        conv3_block(psB[:, 0:256], [sa_feat[:]], w51, 1792, W=256)
        nc.vector.tensor_scalar(sa_conv[:, slB], psB[:, 0:256], b51[:], 0.0,
                                op0=OP.add, op1=OP.max)
        nc.vector.tensor_add(feat_sum[:, slB], sa_conv[:, slB],
                             sc_conv[:, slB])
        t_c8(1536, 0)
        t_c8(1536, 1)
        t_c8(1536, 2)
        t_c8(1536, 3)

    nc.compile()
    return nc


_NC = None


def _get_nc():
    global _NC
    if _NC is None:
        _NC = _build_module()
    return _NC


def _zslot(w):  # [128, C] f32 -> [128, 3, C] fp8 = [w*SC | 0 | w*SC]
    z = np.zeros((128, 3, w.shape[1]), NPF8)
    ws = (w * SC).astype(NPF8)
    z[:, 0, :] = ws
    z[:, 2, :] = ws
    return z


def _prep_inputs(inputs):
    """Host-side: fold BN into conv weights, transpose to lhsT layouts,
    cast matmul operands to bf16. Returns (shared_map, per_core_x)."""
    f32 = np.float32

    def fold(w, g, b, m, v):
        s = (g / np.sqrt(v + EPS)).astype(f32)
        return (w * s[:, None, None]).astype(f32), (b - m * s).astype(f32)

    w5a, b5a = fold(inputs['c5a_w'], inputs['c5a_g'], inputs['c5a_b'],
                    inputs['c5a_m'], inputs['c5a_v'])
    w5c, b5c = fold(inputs['c5c_w'], inputs['c5c_g'], inputs['c5c_b'],
                    inputs['c5c_m'], inputs['c5c_v'])
    w51, b51 = fold(inputs['c51_w'], inputs['c51_g'], inputs['c51_b'],
                    inputs['c51_m'], inputs['c51_v'])
    w52, b52 = fold(inputs['c52_w'], inputs['c52_g'], inputs['c52_b'],
                    inputs['c52_m'], inputs['c52_v'])

    def big_lhsT(w):  # [128, 512, 3] -> [p, chunk*3+tap, c] = [128, 12, 128]
        return np.ascontiguousarray(
            w.reshape(128, 4, 128, 3).transpose(2, 1, 3, 0)
        ).reshape(128, 12, 128)

    def small_lhsT(w):  # [128, 128, 3] -> [p, tap, c] = [128, 3, 128]
        return np.ascontiguousarray(w.transpose(1, 2, 0))

    pa = float(np.asarray(inputs['pa_alpha']).reshape(-1)[0])
    ca = float(np.asarray(inputs['ca_alpha']).reshape(-1)[0])

    shared = {
        'w5a': big_lhsT(w5a).astype(NPBF),
        'b5a': b5a.reshape(128, 1),
        'w5c': big_lhsT(w5c).astype(NPBF),
        'b5c': b5c.reshape(128, 1),
        'wq3': _zslot(inputs['qw'][:, :, 0].T.astype(f32)),
        'wk3': _zslot(inputs['kw'][:, :, 0].T.astype(f32)),
        'bq': np.asarray(inputs['qb']).reshape(CQ, 1).astype(f32),
        'bk': np.asarray(inputs['kb']).reshape(CQ, 1).astype(f32),
        'wv3': _zslot(inputs['vw'][:, :, 0].T.astype(f32)),
        'w51': small_lhsT(w51).astype(NPBF),
        'b51': b51.reshape(128, 1),
        'w52': small_lhsT(w52).astype(NPBF),
        'b52': b52.reshape(128, 1),
        'w8': np.ascontiguousarray(
            inputs['c8_w'][:, :, 0].reshape(4, 128, 128).transpose(2, 0, 1)
        ).astype(NPBF),
        'b8': np.ascontiguousarray(
            inputs['c8_b'].reshape(4, 128).T).astype(f32),
        'alpa': np.full((128, 1), pa, f32),
        'abpa': (pa * np.asarray(inputs['vb'])).reshape(128, 1).astype(f32),
        'alca': np.full((128, 1), ca, f32),
    }
    shared = {k: np.ascontiguousarray(v) for k, v in shared.items()}

    x = np.asarray(inputs['x'])  # [8, 512, 2048]
    per_core_x = [
        np.ascontiguousarray(
            x[b].reshape(4, 128, P).transpose(1, 0, 2).astype(NPBF))
        for b in range(NCORES)
    ]
    return shared, per_core_x


def kernel(**inputs) -> np.ndarray:
    inputs = {k: np.asarray(v) for k, v in inputs.items()}
    nc = _get_nc()
    shared, per_core_x = _prep_inputs(inputs)
    in_maps = [dict(shared, x=per_core_x[b]) for b in range(NCORES)]
    last_err = None
    for _attempt in range(3):
        try:
            res = run_bass_kernel_spmd(nc, in_maps,
                                       core_ids=list(range(NCORES)))
            break
        except Exception as e:  # transient device errors: retry
            last_err = e
            import time as _time
            _time.sleep(2.0)
    else:
        raise last_err
    out = np.stack([res.results[b]['out'].reshape(COUT, P)
                    for b in range(NCORES)])
    return out.astype(np.float32)



# revision 35
# speedup vs baseline: 1.1401x; 1.0056x over previous
"""DualAttention (position attention + channel attention) Trainium2 kernel.

Data-parallel over batch: 8 samples -> 8 NeuronCores, weights replicated.
All heavy matmuls run in bf16 (f32 PSUM accumulation); softmax math,
residual adds and the final output stay f32.

Self-contained: shapes/sharding hardcoded, no sibling imports.
"""

import numpy as np
import ml_dtypes
from contextlib import ExitStack

import concourse.bass as bass
import concourse.tile as tile
from concourse import bacc, mybir
from concourse.bass_utils import run_bass_kernel_spmd
from concourse.masks import make_identity

F32 = mybir.dt.float32
BF16 = mybir.dt.bfloat16
F8 = mybir.dt.float8e4
DR = mybir.MatmulPerfMode.DoubleRow
AF = mybir.ActivationFunctionType
OP = mybir.AluOpType
AX = mybir.AxisListType
NPBF = ml_dtypes.bfloat16
NPF8 = ml_dtypes.float8_e4m3

EPS = 1e-5
SC = 16.0     # fp8 weight scale (keeps w*16 in e4m3 normal range)
RSC = 1.0 / SC
SCW = 32.0    # split-fp8 conv weight scale; feat1/feat2_f kept scaled
RSCW = 1.0 / SCW
P = 2048      # positions
CIN = 512     # input channels (4 chunks of 128)
CI = 128      # inner channels
CQ = 16       # q/k channels
COUT = 512    # output channels (4 chunks of 128)
NCORES = 8
NJC = P // 128   # 16 j-chunks / p-subtiles


def _build_module():
    nc = bacc.Bacc("TRN2", target_bir_lowering=False, debug=False,
                   num_devices=NCORES)

    # ---------------- DRAM I/O ----------------
    dxh = nc.dram_tensor("xh", [128, 4, P], F8, kind="ExternalInput")
    dxl = nc.dram_tensor("xl", [128, 4, P], F8, kind="ExternalInput")
    dw5ah = nc.dram_tensor("w5ah", [128, 12, 128], F8, kind="ExternalInput")
    dw5al = nc.dram_tensor("w5al", [128, 12, 128], F8, kind="ExternalInput")
    db5a = nc.dram_tensor("b5a", [128, 1], F32, kind="ExternalInput")
    dw5ch = nc.dram_tensor("w5ch", [128, 12, 128], F8, kind="ExternalInput")
    dw5cl = nc.dram_tensor("w5cl", [128, 12, 128], F8, kind="ExternalInput")
    db5c = nc.dram_tensor("b5c", [128, 1], F32, kind="ExternalInput")
    dwq3 = nc.dram_tensor("wq3", [128, 3, CQ], F8, kind="ExternalInput")
    dwk3 = nc.dram_tensor("wk3", [128, 3, CQ], F8, kind="ExternalInput")
    dbq = nc.dram_tensor("bq", [CQ, 1], F32, kind="ExternalInput")
    dbk = nc.dram_tensor("bk", [CQ, 1], F32, kind="ExternalInput")
    dwv3 = nc.dram_tensor("wv3", [128, 3, 128], F8, kind="ExternalInput")
    dw51 = nc.dram_tensor("w51", [128, 3, 128], BF16, kind="ExternalInput")
    db51 = nc.dram_tensor("b51", [128, 1], F32, kind="ExternalInput")
    dw52 = nc.dram_tensor("w52", [128, 3, 128], BF16, kind="ExternalInput")
    db52 = nc.dram_tensor("b52", [128, 1], F32, kind="ExternalInput")
    dw8 = nc.dram_tensor("w8", [128, 4, 128], BF16, kind="ExternalInput")
    db8 = nc.dram_tensor("b8", [128, 4], F32, kind="ExternalInput")
    dalpa = nc.dram_tensor("alpa", [128, 1], F32, kind="ExternalInput")
    dabpa = nc.dram_tensor("abpa", [128, 1], F32, kind="ExternalInput")
    dalca = nc.dram_tensor("alca", [128, 1], F32, kind="ExternalInput")
    dout = nc.dram_tensor("out", [4, 128, P], BF16, kind="ExternalOutput")

    with tile.TileContext(nc) as tc, ExitStack() as ctx:
        const = ctx.enter_context(tc.tile_pool(name="const", bufs=1))
        feats = ctx.enter_context(tc.tile_pool(name="feats", bufs=1))
        expsp = ctx.enter_context(tc.tile_pool(name="expsp", bufs=32))
        outp = ctx.enter_context(tc.tile_pool(name="outp", bufs=2))
        smallp = ctx.enter_context(tc.tile_pool(name="smallp", bufs=4))
        # PSUM: st 2x[128,2048]bf16 (4 banks) + cc 2x[128,512]f32 (2 banks)
        #       + tp 1x[128,128]f32 (1 bank) + e2 1x[128,128]f32 (1 bank)
        pst = ctx.enter_context(tc.tile_pool(name="pst", bufs=2, space="PSUM"))
        pcc = ctx.enter_context(tc.tile_pool(name="pcc", bufs=2, space="PSUM"))
        ptp = ctx.enter_context(tc.tile_pool(name="ptp", bufs=1, space="PSUM"))
        pe2 = ctx.enter_context(tc.tile_pool(name="pe2", bufs=1, space="PSUM"))

        # ---------------- constants in ----------------
        _dma_rr = [nc.sync, nc.sync]
        _dma_i = [0]

        def cload(name, shape, dtype, dram):
            t = const.tile(shape, dtype, tag=name)
            eng = _dma_rr[_dma_i[0] % len(_dma_rr)]
            _dma_i[0] += 1
            eng.dma_start(t[:], dram[:])
            return t

        w5ah = const.tile([128, 12, 128], F8, tag="w5ah")
        w5al = const.tile([128, 12, 128], F8, tag="w5al")
        x8h = const.tile([128, 4, P], F8, tag="x8h")
        x8l = const.tile([128, 4, P], F8, tag="x8l")
        # hi operands first so the hi*hi conv pass can start ASAP; each
        # 256-col psum group still waits for its lo pieces before epilogue
        nc.sync.dma_start(w5ah[:], dw5ah[:])
        nc.gpsimd.dma_start(x8h[:, :, 0:516], dxh[:, :, 0:516])
        nc.sync.dma_start(w5al[:], dw5al[:])
        nc.gpsimd.dma_start(x8l[:, :, 0:516], dxl[:, :, 0:516])
        b5a = cload("b5a", [128, 1], F32, db5a)
        xsplit = [516, 1028, 1540, 2048]
        for r in range(3):
            nc.sync.dma_start(x8h[:, :, xsplit[r]:xsplit[r + 1]],
                              dxh[:, :, xsplit[r]:xsplit[r + 1]])
            nc.sync.dma_start(x8l[:, :, xsplit[r]:xsplit[r + 1]],
                              dxl[:, :, xsplit[r]:xsplit[r + 1]])
        wq3 = cload("wq3", [128, 3, CQ], F8, dwq3)
        wk3 = cload("wk3", [128, 3, CQ], F8, dwk3)
        bq = cload("bq", [CQ, 1], F32, dbq)
        bk = cload("bk", [CQ, 1], F32, dbk)
        wv3 = cload("wv3", [128, 3, 128], F8, dwv3)
        abpa = cload("abpa", [128, 1], F32, dabpa)
        w5ch = cload("w5ch", [128, 12, 128], F8, dw5ch)
        w5cl = cload("w5cl", [128, 12, 128], F8, dw5cl)
        b5c = cload("b5c", [128, 1], F32, db5c)
        w51 = cload("w51", [128, 3, 128], BF16, dw51)
        b51 = cload("b51", [128, 1], F32, db51)
        w52 = cload("w52", [128, 3, 128], BF16, dw52)
        b52 = cload("b52", [128, 1], F32, db52)
        w8 = cload("w8", [128, 4, 128], BF16, dw8)
        b8 = cload("b8", [128, 4], F32, db8)
        alpa = cload("alpa", [128, 1], F32, dalpa)
        alca = cload("alca", [128, 1], F32, dalca)

        ident = const.tile([128, 128], BF16, tag="ident")
        make_identity(nc, ident[:])

        # persistent feature tiles
        feat1_f = feats.tile([128, P], F32, tag="feat1_f")
        feat1_b = feats.tile([128, P], F8, tag="feat1_b")
        feat1_a = feats.tile([128, P], F32, tag="feat1_a")  # feat1 + alpha*vb
        feat2_f = feats.tile([128, P], F32, tag="feat2_f")
        feat2_b = feats.tile([128, P], BF16, tag="feat2_b")
        q8 = feats.tile([CQ, P], F8, tag="q8")
        kz = feats.tile([CQ, 3, P], F8, tag="kz")   # [k | 0 | k] pair slots
        vt_all = feats.tile([128, NJC, 130], F8, tag="vt_all")
        f2t_all = feats.tile([128, NJC, 128], BF16, tag="f2t_all")
        sa_feat = feats.tile([128, P], BF16, tag="sa_feat")
        sc_feat = feats.tile([128, P], BF16, tag="sc_feat")
        sa_conv = feats.tile([128, P], BF16, tag="sa_conv")
        sc_conv = feats.tile([128, P], BF16, tag="sc_conv")
        feat_sum = feats.tile([128, P], BF16, tag="feat_sum")

        # ---------------- helpers ----------------
        def conv3_block(psum, rhs2d_list, w_sb, b0, W=512):
            """3-tap conv over output cols [b0, b0+W) into psum [128,W].
            rhs2d_list: list of [128,P] source APs (cin chunks).
            w_sb: [128, 3*nchunks, 128] lhsT per (chunk, tap)."""
            nch = len(rhs2d_list)
            first = True
            for s in (0, -1, 1):
                ol = max(b0, 1) if s == -1 else b0
                oh = min(b0 + W, P - 1) if s == 1 else b0 + W
                for c in range(nch):
                    last = (s == 1 and c == nch - 1)
                    nc.tensor.matmul(
                        psum[:, ol - b0:oh - b0],
                        w_sb[:, c * 3 + (s + 1), :],
                        rhs2d_list[c][:, ol + s:oh + s],
                        start=first, stop=last)
                    first = False

        def conv3_dr(psum, wh, wl, b0, W=256):
            """split-fp8 3-tap conv over cols [b0, b0+W), W<=256 (moving
            free = 2W). Passes hi*hi, hi*lo, lo*hi; chunk-pairs ride the
            DoubleRow pair dim. wh/wl: [128, 12, 128], dim1=(cp*3+tap)*2+pr."""
            first = True
            for pi, (wa, xb) in enumerate(((wh, x8h), (wh, x8l), (wl, x8h))):
                for cp in range(2):
                    for s in (0, -1, 1):
                        ol = max(b0, 1) if s == -1 else b0
                        oh = min(b0 + W, P - 1) if s == 1 else b0 + W
                        last = (pi == 2 and cp == 1 and s == 1)
                        nc.tensor.matmul(
                            psum[:, ol - b0:oh - b0],
                            wa[:, (cp * 3 + s + 1) * 2:
                               (cp * 3 + s + 1) * 2 + 2, :],
                            xb[:, 2 * cp:2 * cp + 2, ol + s:oh + s],
                            start=first, stop=last, perf_mode=DR)
                        first = False


        # warm the ACT exp table off the critical path (first Exp use
        # triggers a ~2.7us table load)
        warm = smallp.tile([128, 1], F32, tag="warm")
        nc.scalar.activation(warm[:], ident[:, 0:1], AF.Exp)
        nc.gpsimd.memset(kz[:, 1, :].bitcast(mybir.dt.uint32), 0)
        nc.vector.memset(vt_all[:, :, 128:130], 1.0)

        # ---- Phase A: conv5a + qk, interleaved so q/k h0 is ready early ---
        def conv5a_block(b):
            # feat1_f/feat1_b stay scaled by SCW (consumers compensate)
            for h in range(2):
                o0 = b * 512 + h * 256
                ps = pcc.tile([128, 256], F32, tag="cc")
                conv3_dr(ps, w5ah, w5al, o0)
                sl = slice(o0, o0 + 256)
                nc.scalar.activation(feat1_f[:, sl], ps[:], AF.Relu,
                                     bias=b5a[:])
                nc.gpsimd.tensor_copy(feat1_b[:, sl], feat1_f[:, sl])

        def qk_half(h):
            # fp8 DoubleRow projections: rhs = adjacent 256-col feat1 pairs,
            # lhsT = [w|0] / [0|w] zero-slot variants
            psq = pst.tile([128, 1024], F32, tag="st")
            psk = pst.tile([128, 1024], F32, tag="st")
            for i in range(4):
                c = 4 * h + i   # 256-col chunk index
                if c % 2 == 0:
                    rh = feat1_b[:, c * 256:(c + 2) * 256]
                    s0, s1 = 0, 2
                else:
                    rh = feat1_b[:, (c - 1) * 256:(c + 1) * 256]
                    s0, s1 = 1, 3
                rhp = rh.rearrange("p (two n) -> p two n", two=2)
                hs = slice(i * 256, (i + 1) * 256)
                nc.tensor.matmul(psq[0:CQ, hs], wq3[:, s0:s1, :], rhp,
                                 start=True, stop=True, perf_mode=DR)
                nc.tensor.matmul(psk[0:CQ, hs], wk3[:, s0:s1, :], rhp,
                                 start=True, stop=True, perf_mode=DR)
            for i in range(2):
                sl = slice(h * 1024 + i * 512, h * 1024 + (i + 1) * 512)
                ph = slice(i * 512, (i + 1) * 512)
                nc.vector.tensor_scalar(q8[0:CQ, sl], psq[0:CQ, ph],
                                        RSC * RSCW, bq[:],
                                        op0=OP.mult, op1=OP.add)
                nc.scalar.activation(kz[0:CQ, 0, sl], psk[0:CQ, ph],
                                     AF.Identity, bias=bk[:],
                                     scale=RSC * RSCW)
                nc.gpsimd.tensor_copy(kz[0:CQ, 2, sl], kz[0:CQ, 0, sl])

        es2 = [None] * 32

        def st_step(t, b):
            es = expsp.tile([128, 1024], F8, tag="expS",
                            name=f"es{t}_{b}")
            es2[t * 4 + b] = es
            ps = pst.tile([128, 1024], F32, tag="st")
            qp = q8[0:CQ, b * 512:(b + 1) * 512].rearrange(
                "k (two n) -> k two n", two=2)
            for j in range(2):
                jc = 2 * t + j
                jb = slice(jc * 128, (jc + 1) * 128)
                o = j * 512
                nc.tensor.matmul(ps[:, o:o + 256], kz[:, 0:2, jb], qp,
                                 start=True, stop=True, perf_mode=DR)
                nc.tensor.matmul(ps[:, o + 256:o + 512], kz[:, 1:3, jb], qp,
                                 start=True, stop=True, perf_mode=DR)
            nc.scalar.activation(es[:], ps[:], AF.Exp)

        conv5a_block(0)
        conv5a_block(1)
        qk_half(0)
        # first two window steps need only q/k cols [0:1024) = qk_half(0):
        # start the exp pipeline while conv5a b2/b3 + qk h1 are in flight
        st_step(0, 0)
        st_step(0, 1)
        conv5a_block(2)
        conv5a_block(3)
        qk_half(1)
        # feat1 + alpha*vb (for the position-attention residual epilogue)
        nc.vector.tensor_scalar(feat1_a[:], feat1_f[:], RSCW, abpa[:],
                                op0=OP.mult, op1=OP.add)

        # ---------------- window filler units -----------------------------
        units = []

        def u_vt(g):
            # vT[p,c] = feat1.T @ wv^T via fp8 DR (adjacent p-block pairs)
            def f():
                ps = pcc.tile([128, 512], F32, tag="cc")
                for i in range(4):
                    sub = g * 4 + i
                    if sub % 2 == 0:
                        lh = feat1_b[:, sub * 128:(sub + 2) * 128]
                        s0, s1 = 0, 2
                    else:
                        lh = feat1_b[:, (sub - 1) * 128:(sub + 1) * 128]
                        s0, s1 = 1, 3
                    lhp = lh.rearrange("p (two n) -> p two n", two=2)
                    nc.tensor.matmul(ps[:, i * 128:(i + 1) * 128], lhp,
                                     wv3[:, s0:s1, :],
                                     start=True, stop=True, perf_mode=DR)
                nc.any.tensor_scalar_mul(
                    vt_all[:, g * 4:(g + 1) * 4, 0:128],
                    ps[:].rearrange("p (s c) -> p s c", s=4), RSC * RSCW)
            return f

        def u_conv5c(hb):
            def f():
                ps = pcc.tile([128, 256], F32, tag="cc")
                conv3_dr(ps, w5ch, w5cl, hb * 256)
                sl = slice(hb * 256, (hb + 1) * 256)
                # feat2_f holds SCW*relu(conv+b); feat2_b is the true scale
                nc.vector.tensor_scalar(feat2_f[:, sl], ps[:], b5c[:], 0.0,
                                        op0=OP.add, op1=OP.max)
                nc.gpsimd.tensor_scalar_mul(feat2_b[:, sl], feat2_f[:, sl],
                                            RSCW)
            return f

        e2_ps = pe2.tile([128, 128], F32, tag="e2")

        def u_f2t(g):
            def f():
                ps = pcc.tile([128, 512], BF16, tag="cc")
                for i in range(4):
                    sub = g * 4 + i
                    nc.tensor.transpose(ps[:, i * 128:(i + 1) * 128],
                                        feat2_b[:, sub * 128:(sub + 1) * 128],
                                        ident[:])
                nc.any.tensor_copy(f2t_all[:, g * 4:(g + 1) * 4, :],
                                   ps[:].rearrange("p (s c) -> p s c", s=4))
                # channel-attention gram accumulation for this group
                for i in range(4):
                    sub = g * 4 + i
                    nc.tensor.matmul(e2_ps[:], f2t_all[:, sub, :],
                                     f2t_all[:, sub, :],
                                     start=(sub == 0), stop=(sub == NJC - 1))
            return f

        attn2 = feats.tile([128, 128], BF16, tag="attn2")
        attn2n = feats.tile([128, 128], BF16, tag="attn2n")
        a2t = feats.tile([128, 128], BF16, tag="a2t")

        def u_softmax2():
            rmin = smallp.tile([128, 1], F32, tag="rmin")
            den2 = smallp.tile([128, 1], F32, tag="den2")
            rden2 = smallp.tile([128, 1], F32, tag="rden2")
            # softmax(max-E) == exp(min-E)/sum: exp(-E + rowmin)
            nc.vector.tensor_reduce(rmin[:], e2_ps[:], axis=AX.X, op=OP.min)
            nc.scalar.activation(attn2[:], e2_ps[:], AF.Exp, bias=rmin[:],
                                 scale=-1.0, accum_out=den2[:])
            nc.vector.reciprocal(rden2[:], den2[:])
            nc.any.tensor_scalar_mul(attn2n[:], attn2[:], rden2[:])
            pt = ptp.tile([128, 128], BF16, tag="tp")
            nc.tensor.transpose(pt[:], attn2n[:], ident[:])
            nc.any.tensor_copy(a2t[:], pt[:])

        def u_out2(b):
            def f():
                ps = pcc.tile([128, 512], F32, tag="cc")
                nc.tensor.matmul(ps[:], a2t[:],
                                 feat2_b[:, b * 512:(b + 1) * 512],
                                 start=True, stop=True)
                # sc_feat = ca_alpha*out2 + feat2
                nc.vector.scalar_tensor_tensor(
                    sc_feat[:, b * 512:(b + 1) * 512], ps[:], alca[:],
                    feat2_b[:, b * 512:(b + 1) * 512], op0=OP.mult, op1=OP.add)
            return f

        def u_c52(b):
            def f():
                ps = pcc.tile([128, 512], F32, tag="cc")
                conv3_block(ps, [sc_feat[:]], w52, b * 512)
                nc.vector.tensor_scalar(sc_conv[:, b * 512:(b + 1) * 512],
                                        ps[:], b52[:], 0.0,
                                        op0=OP.add, op1=OP.max)
            return f

        def u_c51w(o0):
            # in-window c51 block: psum from cc, relu+add on DVE (ACT is the
            # window bottleneck); needs sa_feat cols <= o0+512+1
            def f():
                sl = slice(o0, o0 + 512)
                ps = pcc.tile([128, 512], F32, tag="cc")
                conv3_block(ps, [sa_feat[:]], w51, o0)
                nc.vector.tensor_scalar(sa_conv[:, sl], ps[:], b51[:], 0.0,
                                        op0=OP.add, op1=OP.max)
                nc.vector.tensor_add(feat_sum[:, sl], sa_conv[:, sl],
                                     sc_conv[:, sl])
            return f

        def u_c8w(o0, co):
            def f():
                sl = slice(o0, o0 + 512)
                p8 = pcc.tile([128, 512], F32, tag="cc")
                nc.tensor.matmul(p8[:], w8[:, co, :], feat_sum[:, sl],
                                 start=True, stop=True)
                ot = outp.tile([128, 512], BF16, tag="out_sb", bufs=6)
                nc.vector.tensor_scalar_add(ot[:], p8[:], b8[:, co:co + 1])
                nc.sync.dma_start(dout[co, :, sl], ot[:])
            return f

        for hb in range(8):
            units.append((u_conv5c(hb), 800))
            if hb < 4:
                units.append((u_vt(hb), 600))
        for g in range(4):
            units.append((u_f2t(g), 600))
        units.append((u_softmax2, 300))
        for b in range(4):
            units.append((u_out2(b), 250))
        for b in range(4):
            units.append((u_c52(b), 700))
        units.append((u_c51w(0), 1000))
        for co in range(4):
            units.append((u_c8w(0, co), 600))
        units.append((u_c51w(512), 1000))
        for co in range(4):
            units.append((u_c8w(512, co), 600))

        # ---------------- AV emitter (used in window + after) -------------
        def emit_av(isub):
            ps = pcc.tile([128, 132], F32, tag="cc")
            off = (isub % 4) * 128
            for jcp in range(8):
                est = es2[jcp * 4 + isub // 4]
                lh = est[:].rearrange("p (two n) -> p two n", two=2)
                nc.tensor.matmul(ps[:, 0:129], lh[:, :, off:off + 128],
                                 vt_all[:, 2 * jcp:2 * jcp + 2, 0:129],
                                 start=(jcp == 0), stop=(jcp == 7),
                                 perf_mode=DR)
            rcol = smallp.tile([128, 1], F32, tag="rcol", bufs=8)
            nc.vector.reciprocal(rcol[:], ps[:, 128:129])
            onrm = smallp.tile([128, 128], BF16, tag="onrm", bufs=4)
            nc.any.tensor_scalar_mul(onrm[:], ps[:, 0:128], rcol[:])
            tpool = ptp if isub % 2 == 0 else pe2
            ttag = "tp" if isub % 2 == 0 else "e2"
            pt = tpool.tile([128, 128], BF16, tag=ttag)
            nc.tensor.transpose(pt[:], onrm[:], ident[:])
            # sa_feat = alpha*outT + (feat1 + alpha*vb)
            nc.vector.scalar_tensor_tensor(
                sa_feat[:, isub * 128:(isub + 1) * 128], pt[:], alpa[:],
                feat1_a[:, isub * 128:(isub + 1) * 128],
                op0=OP.mult, op1=OP.add)

        # ---------------- Phase B: S^T + exp window -----------------------
        # S_T[j, i] = sum_d k[d,j] q[d,i]; exp -> expS (fp8, DoubleRow).
        # es2[t*4+b]: [128, 0:512]=expS[2t][:, b*512:], [512:]=expS[2t+1].
        order = [(t, b) for t in range(4) for b in range(2)
                 if (t, b) not in ((0, 0), (0, 1))]
        seen = set(order) | {(0, 0), (0, 1)}
        for b in range(4):
            for t in range(8):
                if (t, b) not in seen:
                    order.append((t, b))
        colcnt = [1, 1, 0, 0]   # (0,0)/(0,1) woven into the head
        av_next = 0
        for (t, b) in order:
            st_step(t, b)
            colcnt[b] += 1
            # keep the PE just behind the ACT exp rate (~1us/step)
            budget = 300.0
            while units and budget > 0:
                f, cost = units.pop(0)
                f()
                budget -= cost
            # AV isubs for completed i-columns ride inside the window
            if av_next < 12 and colcnt[av_next // 4] == 8:
                emit_av(av_next)
                av_next += 1
        # avs 12/13 first: their early MMs depend on es2 tiles finished
        # several window steps ago, so they overlap the window tail; the
        # few leftover units follow in the stream.
        emit_av(12)
        emit_av(13)
        while units:
            units.pop(0)[0]()

        # ------- Phase C/D: AV isubs 8..15 + tail woven in ----------------
        def t_conv(o0, W=512):
            """c51 cols [o0, o0+W) -> feat_sum (ACT relu: ACT is idle here)."""
            sl = slice(o0, o0 + W)
            ps = pst.tile([128, 512], F32, tag="st")
            conv3_block(ps[:, 0:W], [sa_feat[:]], w51, o0, W=W)
            nc.scalar.activation(sa_conv[:, sl], ps[:, 0:W], AF.Relu,
                                 bias=b51[:])
            nc.vector.tensor_add(feat_sum[:, sl], sa_conv[:, sl],
                                 sc_conv[:, sl])

        def t_c8(o0, co, W=512):
            sl = slice(o0, o0 + W)
            p8 = pst.tile([128, 512], F32, tag="st")
            nc.tensor.matmul(p8[:, 0:W], w8[:, co, :], feat_sum[:, sl],
                             start=True, stop=True)
            ot = outp.tile([128, 512], BF16, tag="out_sb", bufs=6)
            nc.any.tensor_scalar_add(ot[:, 0:W], p8[:, 0:W], b8[:, co:co + 1])
            nc.sync.dma_start(dout[co, :, sl], ot[:, 0:W])

        # c51 cols [o, o+W) need sa_feat cols <= o+W, i.e. isubs <= (o+W)/128
        # (isubs 0..11 completed inside the window)
        t_conv(1024)
        emit_av(14)
        t_c8(1024, 0)
        emit_av(15)
        t_c8(1024, 1)
        t_c8(1024, 2)
        t_c8(1024, 3)
        # final c51 block in engine-parallel halves: ACT does one relu while
        # DVE does the other; adds on DVE/gpsimd — shortens the last chain
        slA = slice(1536, 1792)
        psA = pst.tile([128, 512], F32, tag="st", name="c51fA")
        conv3_block(psA[:, 0:256], [sa_feat[:]], w51, 1536, W=256)
        nc.scalar.activation(sa_conv[:, slA], psA[:, 0:256], AF.Relu,
                             bias=b51[:])
        nc.vector.tensor_add(feat_sum[:, slA], sa_conv[:, slA],
                             sc_conv[:, slA])
        slB = slice(1792, 2048)
        psB = pcc.tile([128, 512], F32, tag="cc", name="c51fB")
        conv3_block(psB[:, 0:256], [sa_feat[:]], w51, 1792, W=256)
        nc.vector.tensor_scalar(sa_conv[:, slB], psB[:, 0:256], b51[:], 0.0,
                                op0=OP.add, op1=OP.max)
        nc.vector.tensor_add(feat_sum[:, slB], sa_conv[:, slB],
                             sc_conv[:, slB])
        t_c8(1536, 0)
        t_c8(1536, 1)
        t_c8(1536, 2)
        t_c8(1536, 3)

    nc.compile()
    return nc


_NC = None


def _get_nc():
    global _NC
    if _NC is None:
        _NC = _build_module()
    return _NC


def _zslot(w):  # [128, C] f32 -> [128, 3, C] fp8 = [w*SC | 0 | w*SC]
    z = np.zeros((128, 3, w.shape[1]), NPF8)
    ws = (w * SC).astype(NPF8)
    z[:, 0, :] = ws
    z[:, 2, :] = ws
    return z


def _prep_inputs(inputs):
    """Host-side: fold BN into conv weights, transpose to lhsT layouts,
    cast matmul operands to bf16. Returns (shared_map, per_core_x)."""
    f32 = np.float32

    def fold(w, g, b, m, v):
        s = (g / np.sqrt(v + EPS)).astype(f32)
        return (w * s[:, None, None]).astype(f32), (b - m * s).astype(f32)

    w5a, b5a = fold(inputs['c5a_w'], inputs['c5a_g'], inputs['c5a_b'],
                    inputs['c5a_m'], inputs['c5a_v'])
    w5c, b5c = fold(inputs['c5c_w'], inputs['c5c_g'], inputs['c5c_b'],
                    inputs['c5c_m'], inputs['c5c_v'])
    w51, b51 = fold(inputs['c51_w'], inputs['c51_g'], inputs['c51_b'],
                    inputs['c51_m'], inputs['c51_v'])
    w52, b52 = fold(inputs['c52_w'], inputs['c52_g'], inputs['c52_b'],
                    inputs['c52_m'], inputs['c52_v'])

    def big_lhsT(w):  # [128, 512, 3] -> [p, chunk*3+tap, c] = [128, 12, 128]
        return np.ascontiguousarray(
            w.reshape(128, 4, 128, 3).transpose(2, 1, 3, 0)
        ).reshape(128, 12, 128)

    def split_pair(w):
        """[128, 12, 128] f32 lhsT -> (hi, lo) fp8 in paired layout
        [p, (cp*3+tap)*2+pair, c]: scale by SCW, pair adjacent chunks."""
        ws = (w * SCW).astype(np.float32)
        hi = ws.astype(NPF8)
        lo = (ws - hi.astype(np.float32)).astype(NPF8)

        def pairup(a):  # dim1 c*3+t -> (cp*3+t)*2+p with c = 2*cp+p
            b = a.reshape(128, 4, 3, 128)
            b = b.reshape(128, 2, 2, 3, 128)
            b = b.transpose(0, 1, 3, 2, 4)
            return np.ascontiguousarray(b).reshape(128, 12, 128)
        return pairup(hi), pairup(lo)

    def small_lhsT(w):  # [128, 128, 3] -> [p, tap, c] = [128, 3, 128]
        return np.ascontiguousarray(w.transpose(1, 2, 0))

    pa = float(np.asarray(inputs['pa_alpha']).reshape(-1)[0])
    ca = float(np.asarray(inputs['ca_alpha']).reshape(-1)[0])

    w5ah, w5al = split_pair(big_lhsT(w5a))
    w5ch, w5cl = split_pair(big_lhsT(w5c))
    shared = {
        'w5ah': w5ah, 'w5al': w5al,
        'b5a': (SCW * b5a).reshape(128, 1),
        'w5ch': w5ch, 'w5cl': w5cl,
        'b5c': (SCW * b5c).reshape(128, 1),
        'wq3': _zslot(inputs['qw'][:, :, 0].T.astype(f32)),
        'wk3': _zslot(inputs['kw'][:, :, 0].T.astype(f32)),
        'bq': np.asarray(inputs['qb']).reshape(CQ, 1).astype(f32),
        'bk': np.asarray(inputs['kb']).reshape(CQ, 1).astype(f32),
        'wv3': _zslot(inputs['vw'][:, :, 0].T.astype(f32)),
        'w51': small_lhsT(w51).astype(NPBF),
        'b51': b51.reshape(128, 1),
        'w52': small_lhsT(w52).astype(NPBF),
        'b52': b52.reshape(128, 1),
        'w8': np.ascontiguousarray(
            inputs['c8_w'][:, :, 0].reshape(4, 128, 128).transpose(2, 0, 1)
        ).astype(NPBF),
        'b8': np.ascontiguousarray(
            inputs['c8_b'].reshape(4, 128).T).astype(f32),
        'alpa': np.full((128, 1), pa, f32),
        'abpa': (pa * np.asarray(inputs['vb'])).reshape(128, 1).astype(f32),
        'alca': np.full((128, 1), ca, f32),
    }
    shared = {k: np.ascontiguousarray(v) for k, v in shared.items()}

    x = np.asarray(inputs['x'])  # [8, 512, 2048]
    per_core_x = []
    for b in range(NCORES):
        xc = np.ascontiguousarray(
            x[b].reshape(4, 128, P).transpose(1, 0, 2)).astype(np.float32)
        xh = xc.astype(NPF8)
        xl = (xc - xh.astype(np.float32)).astype(NPF8)
        per_core_x.append((xh, xl))
    return shared, per_core_x


def kernel(**inputs) -> np.ndarray:
    inputs = {k: np.asarray(v) for k, v in inputs.items()}
    nc = _get_nc()
    shared, per_core_x = _prep_inputs(inputs)
    in_maps = [dict(shared, xh=per_core_x[b][0], xl=per_core_x[b][1])
               for b in range(NCORES)]
    last_err = None
    for _attempt in range(3):
        try:
            res = run_bass_kernel_spmd(nc, in_maps,
                                       core_ids=list(range(NCORES)))
            break
        except Exception as e:  # transient device errors: retry
            last_err = e
            import time as _time
            _time.sleep(2.0)
    else:
        raise last_err
    out = np.stack([res.results[b]['out'].reshape(COUT, P)
                    for b in range(NCORES)])
    return out.astype(np.float32)

